# revision 1
# baseline (speedup 1.0000x reference)
"""OGRENet GNN message-passing kernel for 8 Trainium2 NeuronCores.

Strategy
--------
Host (numpy, cheap index plumbing only):
  * u2 = u @ Wsel + bsel  (64x256, negligible FLOPs)
  * sort edges by destination node (`row`), split into 8 contiguous chunks at
    node boundaries (2500 nodes per core) -> each core owns a contiguous node
    range and ALL edges that scatter into it => no cross-core reduction.
  * pack per-core feature-major edge inputs ein.T = [x[col]; x[row]; e_attr;
    u2[batch[row]]] (fp16), per-edge 1/count scales, window-relative row ids.

Device (per core, identical program, different data):
  * edge MLP (275->1024x4->512) + node MLP1 (521->512->512) as fp16 matmuls,
    feature-major activations [feat_part, edge_free], fp32 PSUM accumulation.
  * node MLP1 layer2 flips to edge-major [128 edges, 512 feat] so the
    segment-sum becomes a matmul: membership tile M[e, n] = (row[e]==n)
    (built on DVE via is_equal against an iota row) contracts edges away:
    agg.T[f, n] += h.T_tile @ M.  1/count is folded into h via the ACT scale.
  * PSUM accumulates each 256-node window over a static range of edge tiles
    (ranges computed from the actual data on host, shared by all cores;
    membership zeroes any edge outside the window, so overlap is harmless).
  * node MLP2 (777->512->1) consumes agg feature-major directly; z -> DRAM.
"""

import os
import sys

import numpy as np

sys.path.insert(0, "/opt/trn_rl_repo")

N_NODES = 20000
N_GRAPHS = 64
U_DIM = 256
E_HID = 1024
E_OUT = 512
N_HID = 512
NC = 8
NPN = N_NODES // NC          # nodes per core (2500)
NPAD = 2560                  # padded nodes per core
WN = 256                     # nodes per segment window
NWIN = NPAD // WN            # 10
P = 128

_CACHE = {}


def _pack_cols(v, T):
    """[T*128] -> [128, T] with col t = v[t*128:(t+1)*128]."""
    return np.ascontiguousarray(v.reshape(T, P).T)


def _build_module(EPAD, win_tiles, h2_bufs):
    """Build the per-core Bass program. win_tiles[w] = (tlo, thi) inclusive."""
    from concourse import bacc, mybir, tile

    T = EPAD // P           # 128-edge tiles
    NB = EPAD // 1024       # 1024-edge blocks
    f16 = mybir.dt.float16
    f32 = mybir.dt.float32
    RELU = mybir.ActivationFunctionType.Relu
    COPY = mybir.ActivationFunctionType.Copy
    IDENT = mybir.ActivationFunctionType.Identity

    nc = bacc.Bacc(None, target_bir_lowering=False, debug=False)

    with tile.TileContext(nc) as tc:
        with (
            tc.tile_pool(name="dram", bufs=1, space="DRAM") as dram,
            tc.tile_pool(name="wres", bufs=1) as wres,
            tc.tile_pool(name="einp", bufs=5) as einp,
            tc.tile_pool(name="actp", bufs=8) as actp,
            tc.tile_pool(name="act4", bufs=4) as act4,
            tc.tile_pool(name="h2p", bufs=h2_bufs) as h2p,
            tc.tile_pool(name="smal", bufs=3) as smal,
            tc.tile_pool(name="aggp", bufs=8) as aggp,
            tc.tile_pool(name="strm", bufs=4) as strm,
            tc.tile_pool(name="bigps", bufs=3, space="PSUM") as bigps,
            tc.tile_pool(name="segps", bufs=4, space="PSUM") as segps,
        ):
            # ---- DRAM I/O -------------------------------------------------
            d_ein = dram.tile([384, EPAD], f16, kind="ExternalInput", name="ein")
            d_relw = dram.tile([P, NWIN * T], f32, kind="ExternalInput", name="relw")
            d_invc = dram.tile([P, T], f32, kind="ExternalInput", name="invc")
            d_xT2 = dram.tile([P, NPAD], f16, kind="ExternalInput", name="xT2")
            d_u2bT = dram.tile([U_DIM, NPAD], f16, kind="ExternalInput", name="u2bT")
            d_iota = dram.tile([P, WN], f32, kind="ExternalInput", name="iota")
            d_n1b2bc = dram.tile([P, 512], f32, kind="ExternalInput", name="n1b2bc")

            wspec = dict(
                eW1p=[384, E_HID],
                n1W1a=[P, N_HID],
                n2W1x=[P, N_HID], n2W1agg=[N_HID, N_HID], n2W1u=[U_DIM, N_HID],
                n2W2=[N_HID, 1],
            )
            d_w = {k: dram.tile(s, f16, kind="ExternalInput", name=k)
                   for k, s in wspec.items()}
            f8 = mybir.dt.float8e4
            for k in ("eW2", "eW3", "eW4"):
                d_w[k] = dram.tile([P, 8192], f8, kind="ExternalInput", name=k)
            d_w["eW5"] = dram.tile([P, 4096], f8, kind="ExternalInput", name="eW5")
            d_w["n1W1b"] = dram.tile([P, 2048], f8, kind="ExternalInput", name="n1W1b")
            d_w["n1W2"] = dram.tile([P, 2048], f8, kind="ExternalInput", name="n1W2")
            bspec = dict(eb1r=[P, 8], eb2r=[P, 8], eb3r=[P, 8], eb4r=[P, 8],
                         eb5r=[P, 4], n1b1r=[P, 4], n2b1r=[P, 4], n2b2r=[1, 1])
            d_b = {k: dram.tile(s, f32, kind="ExternalInput", name=k)
                   for k, s in bspec.items()}
            d_z = dram.tile([1, NPAD], f32, kind="ExternalOutput", name="zout")

            names = dict(ein=d_ein.name, relw=d_relw.name, invc=d_invc.name,
                         xT2=d_xT2.name, u2bT=d_u2bT.name, iota=d_iota.name,
                         n1b2bc=d_n1b2bc.name, zout=d_z.name)
            names.update({k: v.name for k, v in d_w.items()})
            names.update({k: v.name for k, v in d_b.items()})

            # ---- resident loads ------------------------------------------
            def load_w(name, nk, width):
                ts = []
                for k in range(nk):
                    t = wres.tile([P, width], f16, name=f"w_{name}_{k}")
                    nc.sync.dma_start(out=t[:], in_=d_w[name][k * P:(k + 1) * P, :])
                    ts.append(t)
                return ts

            W1 = load_w("eW1p", 3, E_HID)

            def load_wdr(name, npair, width):
                ts = []
                for q in range(npair):
                    t = wres.tile([P, 2, width], f8, name=f"w_{name}_{q}")
                    for j in range(2):
                        nc.sync.dma_start(
                            out=t[:, j, :],
                            in_=d_w[name][:, (q * 2 + j) * width:
                                          (q * 2 + j + 1) * width])
                    ts.append(t)
                return ts

            W2 = load_wdr("eW2", 4, E_HID)
            W3 = load_wdr("eW3", 4, E_HID)
            W4 = load_wdr("eW4", 4, E_HID)
            W5 = load_wdr("eW5", 4, E_OUT)
            W6a = load_w("n1W1a", 1, N_HID)
            W6b = load_wdr("n1W1b", 2, N_HID)
            W7 = load_wdr("n1W2", 2, N_HID)
            W8x = load_w("n2W1x", 1, N_HID)
            W8a = load_w("n2W1agg", 4, N_HID)
            W8u = load_w("n2W1u", 2, N_HID)
            W9 = load_w("n2W2", 4, 1)

            B = {}
            for k, s in bspec.items():
                t = wres.tile(s, f32, name=f"b_{k}")
                nc.sync.dma_start(out=t[:], in_=d_b[k][:])
                B[k] = t
            relw = wres.tile([P, NWIN * T], f32, name="relw_sb")
            nc.sync.dma_start(out=relw[:], in_=d_relw[:])
            invc = wres.tile([P, T], f32, name="invc_sb")
            nc.sync.dma_start(out=invc[:], in_=d_invc[:])
            iota = wres.tile([P, WN], f32, name="iota_sb")
            nc.sync.dma_start(out=iota[:], in_=d_iota[:])
            n1b2bc = wres.tile([P, 512], f32, name="n1b2bc_sb")
            nc.sync.dma_start(out=n1b2bc[:], in_=d_n1b2bc[:])
            xT2 = wres.tile([P, NPAD], f16, name="xT2_sb")
            nc.sync.dma_start(out=xT2[:], in_=d_xT2[:])

            # ---- helpers --------------------------------------------------
            def fm_layer(ins, Wt, bias, nM, width, act, out_pool, tag):
                """feature-major layer: out[m] [128, width_edges]"""
                outs = []
                for m in range(nM):
                    ps = bigps.tile([P, 512], mybir.dt.float32, name="ps_big")
                    pss = ps[:, :width]
                    for ki, (it, wt) in enumerate(zip(ins, Wt)):
                        nc.tensor.matmul(
                            out=pss, lhsT=wt[:, m * P:(m + 1) * P], rhs=it,
                            start=(ki == 0), stop=(ki == len(ins) - 1))
                    ot = out_pool.tile([P, width], f16, name=tag)
                    nc.scalar.activation(ot[:], pss, act, bias=bias[:, m:m + 1])
                    outs.append(ot)
                return outs

            DR = mybir.MatmulPerfMode.DoubleRow
            INV64 = 1.0 / 64.0

            def dr_layer(pin, Wp, bias, scale, nM, act, pair_out, tag):
                """fp8 DoubleRow layer: pin = pair tiles [128,2,512]."""
                outs = []
                pt = None
                for m in range(nM):
                    ps = bigps.tile([P, 512], mybir.dt.float32, name="ps_big")
                    for q in range(len(Wp)):
                        nc.tensor.matmul(
                            out=ps[:], lhsT=Wp[q][:, :, m * P:(m + 1) * P],
                            rhs=pin[q][:, :, :], start=(q == 0),
                            stop=(q == len(Wp) - 1), perf_mode=DR)
                    if pair_out:
                        if m % 2 == 0:
                            pt = actp.tile([P, 2, 512], f8, name=tag)
                            outs.append(pt)
                        nc.scalar.activation(pt[:, m % 2, :], ps[:], act,
                                             bias=bias[:, m:m + 1], scale=scale)
                    else:
                        ot = actp.tile([P, 512], f16, name=tag)
                        nc.scalar.activation(ot[:], ps[:], act,
                                             bias=bias[:, m:m + 1], scale=scale)
                        outs.append(ot)
                return outs

            # h2 tiles by global tile index
            h2_tiles = {}

            def emit_block(b):
                e0 = b * 1024
                ein = []
                for k in range(3):
                    t = einp.tile([P, 1024], f16, name="ein_t")
                    nc.sync.dma_start(out=t[:], in_=d_ein[k * P:(k + 1) * P, e0:e0 + 1024])
                    ein.append(t)
                for h in range(2):
                    hs = slice(h * 512, (h + 1) * 512)
                    einh = [e[:, hs] for e in ein]
                    a1p = []
                    pt = None
                    for m in range(8):
                        ps = bigps.tile([P, 512], mybir.dt.float32, name="ps_big")
                        for ki in range(3):
                            nc.tensor.matmul(
                                out=ps[:], lhsT=W1[ki][:, m * P:(m + 1) * P],
                                rhs=einh[ki], start=(ki == 0), stop=(ki == 2))
                        if m % 2 == 0:
                            pt = actp.tile([P, 2, 512], f8, name="pairA")
                            a1p.append(pt)
                        nc.scalar.activation(pt[:, m % 2, :], ps[:], RELU,
                                             bias=B["eb1r"][:, m:m + 1])
                    a2p = dr_layer(a1p, W2, B["eb2r"], INV64, 8, RELU, True, "pairB")
                    a3p = dr_layer(a2p, W3, B["eb3r"], INV64, 8, RELU, True, "pairA")
                    a4p = dr_layer(a3p, W4, B["eb4r"], INV64, 8, RELU, True, "pairB")
                    e5p = dr_layer(a4p, W5, B["eb5r"], INV64, 4, IDENT, True, "pairC")
                    # n1 L1: fp16 x-part (x64 weights) + fp8 DR e-part
                    h1p = []
                    pt = None
                    for m in range(4):
                        ps = bigps.tile([P, 512], mybir.dt.float32, name="ps_big")
                        nc.tensor.matmul(
                            out=ps[:], lhsT=W6a[0][:, m * P:(m + 1) * P],
                            rhs=einh[0], start=True, stop=False)
                        for q in range(2):
                            nc.tensor.matmul(
                                out=ps[:], lhsT=W6b[q][:, :, m * P:(m + 1) * P],
                                rhs=e5p[q][:, :, :], start=False, stop=(q == 1),
                                perf_mode=DR)
                        if m % 2 == 0:
                            pt = actp.tile([P, 2, 512], f8, name="pairD")
                            h1p.append(pt)
                        nc.scalar.activation(pt[:, m % 2, :], ps[:], RELU,
                                             bias=B["n1b1r"][:, m:m + 1],
                                             scale=INV64)
                    # n1 layer2 -> edge-major h2 per 128-edge subtile, then
                    # bias-add (DVE, x64 domain) + relu*(invc/64) (ACT) -> fp16
                    for s in range(4):
                        t_glob = b * 8 + h * 4 + s
                        ps = bigps.tile([P, 512], mybir.dt.float32, name="ps_big")
                        pss = ps[:, :512]
                        for q in range(2):
                            nc.tensor.matmul(
                                out=pss, lhsT=h1p[q][:, :, s * P:(s + 1) * P],
                                rhs=W7[q][:, :, :], start=(q == 0), stop=(q == 1),
                                perf_mode=DR)
                        tmp = smal.tile([P, 512], f16, name="tmp16")
                        nc.vector.tensor_tensor(out=tmp[:], in0=pss, in1=n1b2bc[:],
                                                op=mybir.AluOpType.add)
                        h2 = h2p.tile([P, 512], f16, name="h2t")
                        nc.scalar.activation(h2[:], tmp[:], RELU,
                                             scale=invc[:, t_glob:t_glob + 1])
                        h2_tiles[t_glob] = h2

            def emit_window(w):
                tlo, thi = win_tiles[w]
                seg = [segps.tile([P, WN], mybir.dt.float32, name="segps_t")
                       for _ in range(4)]
                tl = list(range(tlo, thi + 1))
                for si, t in enumerate(tl):
                    memb = smal.tile([P, WN], f16, name="memb")
                    nc.vector.tensor_scalar(
                        out=memb[:], in0=iota[:],
                        scalar1=relw[:, w * T + t:w * T + t + 1], scalar2=None,
                        op0=mybir.AluOpType.is_equal)
                    h2 = h2_tiles[t]
                    for fc in range(4):
                        nc.tensor.matmul(
                            out=seg[fc][:], lhsT=h2[:, fc * P:(fc + 1) * P],
                            rhs=memb[:], start=(si == 0), stop=(si == len(tl) - 1))
                agg = []
                for fc in range(4):
                    at = aggp.tile([P, WN], f16, name="aggt")
                    nc.scalar.activation(at[:], seg[fc][:], COPY)
                    agg.append(at)
                # ---- node MLP2 on this 256-node window ----
                n0 = w * WN
                u2c = []
                for k in range(2):
                    t = strm.tile([P, WN], f16, name="u2c")
                    nc.sync.dma_start(out=t[:], in_=d_u2bT[k * P:(k + 1) * P,
                                                          n0:n0 + WN])
                    u2c.append(t)
                xin = [xT2[:, n0:n0 + WN]] + [a[:] for a in agg] + \
                      [u2c[0][:], u2c[1][:]]
                Win = [W8x[0]] + W8a + W8u
                z1 = []
                for m in range(4):
                    ps = bigps.tile([P, 512], mybir.dt.float32, name="ps_big")
                    pss = ps[:, :WN]
                    for ki in range(7):
                        nc.tensor.matmul(out=pss, lhsT=Win[ki][:, m * P:(m + 1) * P],
                                         rhs=xin[ki], start=(ki == 0), stop=(ki == 6))
                    zt = smal.tile([P, WN], f16, name="z1t")
                    nc.scalar.activation(zt[:], pss, RELU,
                                         bias=B["n2b1r"][:, m:m + 1])
                    z1.append(zt)
                ps = bigps.tile([P, 512], mybir.dt.float32, name="ps_big")
                pss = ps[:1, :WN]
                for ki in range(4):
                    nc.tensor.matmul(out=pss, lhsT=W9[ki][:], rhs=z1[ki][:],
                                     start=(ki == 0), stop=(ki == 3))
                zo = smal.tile([1, WN], f32, name="zot")
                nc.scalar.activation(zo[:], pss, IDENT, bias=B["n2b2r"][:])
                nc.sync.dma_start(out=d_z[:, n0:n0 + WN], in_=zo[:])

            # window w ready once tile win_tiles[w][1] is produced
            ready = {}
            for w in range(NWIN):
                b_ready = min(NB - 1, win_tiles[w][1] // 8)
                ready.setdefault(b_ready, []).append(w)
            for b in range(NB):
                emit_block(b)
                for w in ready.get(b, []):
                    emit_window(w)

    nc.compile()
    return nc, names


def kernel(x, edge_attr, u, edge_index, batch, Wsel, bsel,
           eW1, eb1, eW2, eb2, eW3, eb3, eW4, eb4, eW5, eb5,
           n1W1, n1b1, n1W2, n1b2, n2W1, n2b1, n2W2, n2b2):
    f32 = np.float32
    f16 = np.float16
    x = np.asarray(x, f32)
    edge_attr = np.asarray(edge_attr, f32)
    u = np.asarray(u, f32)
    edge_index = np.asarray(edge_index)
    batch = np.asarray(batch)
    ws = {k: np.asarray(v, f32) for k, v in dict(
        Wsel=Wsel, bsel=bsel, eW1=eW1, eb1=eb1, eW2=eW2, eb2=eb2, eW3=eW3,
        eb3=eb3, eW4=eW4, eb4=eb4, eW5=eW5, eb5=eb5, n1W1=n1W1, n1b1=n1b1,
        n1W2=n1W2, n1b2=n1b2, n2W1=n2W1, n2b1=n2b1, n2W2=n2W2, n2b2=n2b2).items()}

    # ---------------- host math (index plumbing + tiny matmul) -------------
    u2 = (u @ ws["Wsel"] + ws["bsel"]).astype(f32)          # [64, 256]
    row = np.asarray(edge_index[0], np.int64)
    col = np.asarray(edge_index[1], np.int64)
    order = np.argsort(row, kind="stable")
    row_s, col_s = row[order], col[order]
    g_s = batch[row_s]
    ea_s = edge_attr[order, 0]
    cnt = np.bincount(row, minlength=N_NODES).astype(f32)
    invc_node = (1.0 / np.maximum(cnt, 1.0)).astype(f32)

    bounds = np.searchsorted(row_s, np.arange(0, N_NODES + 1, NPN))
    e_cnt = np.diff(bounds)
    EPAD = int(-(-int(e_cnt.max()) // 1024) * 1024)
    T = EPAD // P

    # per-window tile ranges (shared across cores) for the static program
    tlo = np.full(NWIN, T - 1, np.int64)
    thi = np.zeros(NWIN, np.int64)
    core_dat = []
    for c in range(NC):
        lo, hi = bounds[c], bounds[c + 1]
        n = hi - lo
        rel = np.full(EPAD, 1e6, f32)
        rel[:n] = (row_s[lo:hi] - NPN * c).astype(f32)
        w_of_edge = np.floor_divide(rel[:n].astype(np.int64), WN)
        for w in range(NWIN):
            idx = np.nonzero(w_of_edge == w)[0]
            if idx.size:
                tlo[w] = min(tlo[w], idx[0] // P)
                thi[w] = max(thi[w], idx[-1] // P)
        core_dat.append((lo, hi, n, rel))
    win_tiles = [(int(tlo[w]), int(max(tlo[w], thi[w]))) for w in range(NWIN)]
    h2_bufs = max(hw - lw + 1 for lw, hw in win_tiles) + 14

    # ---------------- per-core input packing --------------------------------
    zpad109 = np.zeros((109, ws["eW1"].shape[1]), f32)
    eW1p = np.concatenate([ws["eW1"][9:18], ws["eW1"][0:9], ws["eW1"][18:19],
                           ws["eW1"][19:275], zpad109], axis=0)
    n1W1a = np.zeros((P, N_HID), f32)
    n1W1a[0:9] = ws["n1W1"][0:9] * 64.0
    n2W1x = np.zeros((P, N_HID), f32)
    n2W1x[0:9] = ws["n2W1"][0:9]

    def br(b, nm):   # bias [nm*128] -> [128, nm]
        return np.ascontiguousarray(b.reshape(nm, P).T).astype(f32)

    import ml_dtypes
    fp8 = ml_dtypes.float8_e4m3

    def packdr(W):   # [K, M] -> [128, (K//128)*M] fp8, x64, (q,j,m) free order
        K, M = W.shape
        Wp = (W * 64.0).reshape(K // 256, 2, P, M)
        return np.ascontiguousarray(
            np.transpose(Wp, (2, 0, 1, 3)).reshape(P, (K // P) * M)).astype(fp8)

    shared = dict(
        eW1p=eW1p.astype(f16), eW2=packdr(ws["eW2"]),
        eW3=packdr(ws["eW3"]), eW4=packdr(ws["eW4"]),
        eW5=packdr(ws["eW5"]), n1W1a=n1W1a.astype(f16),
        n1W1b=packdr(ws["n1W1"][9:521]), n1W2=packdr(ws["n1W2"]),
        n2W1x=n2W1x.astype(f16), n2W1agg=ws["n2W1"][9:521].astype(f16),
        n2W1u=ws["n2W1"][521:777].astype(f16), n2W2=ws["n2W2"].astype(f16),
        eb1r=br(ws["eb1"], 8), eb2r=br(ws["eb2"], 8), eb3r=br(ws["eb3"], 8),
        eb4r=br(ws["eb4"], 8), eb5r=br(ws["eb5"], 4), n1b1r=br(ws["n1b1"], 4),
        n2b1r=br(ws["n2b1"], 4), n2b2r=ws["n2b2"].reshape(1, 1).astype(f32),
        iota=np.tile(np.arange(WN, dtype=f32), (P, 1)),
        n1b2bc=np.tile(ws["n1b2"].astype(f32) * 64.0, (P, 1)),
    )

    in_maps = []
    for c in range(NC):
        lo, hi, n, rel = core_dat[c]
        ein = np.zeros((384, EPAD), f16)
        ein[0:9, :n] = x[col_s[lo:hi]].T
        ein[9:18, :n] = x[row_s[lo:hi]].T
        ein[18, :n] = ea_s[lo:hi]
        ein[19:275, :n] = u2[g_s[lo:hi]].T
        relw = np.empty((P, NWIN * T), f32)
        for w in range(NWIN):
            relw[:, w * T:(w + 1) * T] = _pack_cols(rel - float(WN) * w, T)
        invc_e = np.ones(EPAD, f32)
        invc_e[:n] = invc_node[row_s[lo:hi]]
        invc_e *= 1.0 / 64.0
        xT2 = np.zeros((P, NPAD), f16)
        xT2[0:9, :NPN] = x[NPN * c:NPN * (c + 1)].T
        u2bT = np.zeros((U_DIM, NPAD), f16)
        u2bT[:, :NPN] = u2[batch[NPN * c:NPN * (c + 1)]].T
        im = dict(shared)
        im.update(ein=ein, relw=relw, invc=_pack_cols(invc_e, T),
                  xT2=xT2, u2bT=u2bT)
        in_maps.append(im)

    # ---------------- build + run ------------------------------------------
    key = (EPAD, tuple(win_tiles))
    if key not in _CACHE:
        _CACHE[key] = _build_module(EPAD, win_tiles, h2_bufs)
    nc, names = _CACHE[key]

    from concourse import bass_utils
    trace = bool(int(os.environ.get("KERNEL_TRACE", "0")))
    if trace:
        try:
            import types
            import antenv
            if not hasattr(antenv, "axon_hooks"):
                mod = types.ModuleType("antenv.axon_hooks")
                mod._hook = None
                mod.set_axon_ntff_profile_hook = lambda h: setattr(mod, "_hook", h)
                mod.get_axon_ntff_profile_hook = lambda: mod._hook
                sys.modules["antenv.axon_hooks"] = mod
                antenv.axon_hooks = mod
                from trn_agent_boot.trn_boot import _ntff_profile_via_ctypes
                mod._hook = _ntff_profile_via_ctypes("/opt/axon/libaxon_pjrt.so")
        except Exception as e:  # profiling is best-effort
            print("ntff hook shim failed:", e)
            trace = False
    real_maps = [{names[k]: v for k, v in im.items()} for im in in_maps]
    res = bass_utils.run_bass_kernel_spmd(
        nc, real_maps, core_ids=list(range(NC)), trace=trace)
    if trace and res.exec_time_ns is not None:
        print(f"HW exec time: {res.exec_time_ns} ns")
        if res.instructions_and_trace:
            print("trace:", res.instructions_and_trace[1])

    out = np.empty(N_NODES, f32)
    for c in range(NC):
        out[NPN * c:NPN * (c + 1)] = res.results[c][names["zout"]][0, :NPN]
    return out



# revision 5
# speedup vs baseline: 1.2086x; 1.2086x over previous
"""OGRENet GNN message-passing kernel for 8 Trainium2 NeuronCores.

Strategy
--------
Host (numpy, cheap index plumbing only):
  * u2 = u @ Wsel + bsel (64x256), G64 = u2 @ eW1[19:275] (64x1024),
    H64 = u2 @ n2W1[521:777] (64x512): the per-edge/node u2 gather becomes a
    64-wide one-hot riding inside the K=128 first-layer matmul.
  * W_combo = eW5 @ n1W1[9:521]: e5 is never materialized (it is only ever
    consumed linearly by node MLP1), fusing two layers into one.
  * sort edges by destination node (`row`), split into 8 contiguous chunks at
    node boundaries -> each core owns a contiguous node range and ALL edges
    that scatter into it => no cross-core reduction.
  * pack per-core feature-major edge inputs ein = [x[col]; x[row]; e_attr;
    onehot(g); 1] (fp16, 128 rows total), per-edge 1/count scales, and
    window-relative row ids.

Device (per core, identical program, different data):
  * edge MLP layers as fp8 DoubleRow matmuls (157 TF/s), fp32 PSUM. Biases and
    relu are applied in a scale-staged way so the quantize ops spread across
    ACT/DVE/GpSimd and the tensor engine never waits:
      a1 = relu(ps)            x1   ACT   (bias via ones-row in ein)
      a2 = max(ps + 64 b2, 0)  x64  DVE   (2-op tensor_scalar)
      a3 = relu(ps/4096 + b3)  x1   ACT
      a4 = max(ps + 64 b4, 0)  x64  DVE
      h1 = relu(ps/64)         x64  ACT   (bias via ones-row in x-part)
      h2 = max((ps+4096 b) * invc/64, 0) fp8 x64  GpSimd add + DVE mul/relu
  * segment-sum as matmul with fp8 DoubleRow: membership pairs M[e,n]
    (is_equal on DVE) contract TWO 128-edge tiles per pass into a
    [feat, 256-node] PSUM window.
  * node MLP2 consumes agg feature-major; u2[batch] + bias enter through
    one-hot/ones rows of the x input tile; z -> DRAM.
"""

import os
import sys

import numpy as np

sys.path.insert(0, "/opt/trn_rl_repo")

N_NODES = 20000
N_GRAPHS = 64
E_HID = 1024
N_HID = 512
NC = 8
NPN = N_NODES // NC          # nodes per core (2500)
NPAD = 2560                  # padded nodes per core
WN = 256                     # nodes per segment window
NWIN = NPAD // WN            # 10
P = 128

_CACHE = {}


def _pack_cols(v, T):
    """[T*128] -> [128, T] with col t = v[t*128:(t+1)*128]."""
    return np.ascontiguousarray(v.reshape(T, P).T)


def _build_module(EPAD, win_tiles, h2_bufs):
    """Build the per-core Bass program. win_tiles[w] = (tlo, thi) inclusive."""
    from concourse import bacc, mybir, tile

    T = EPAD // P           # 128-edge tiles
    NB = EPAD // 1024       # 1024-edge blocks
    win_pairs = [(tl // 2, th // 2) for tl, th in win_tiles]
    f16 = mybir.dt.float16
    f32 = mybir.dt.float32
    f8 = mybir.dt.float8e4
    RELU = mybir.ActivationFunctionType.Relu
    COPY = mybir.ActivationFunctionType.Copy
    ADD = mybir.AluOpType.add
    MULT = mybir.AluOpType.mult
    MAX = mybir.AluOpType.max
    ISEQ = mybir.AluOpType.is_equal
    DR = mybir.MatmulPerfMode.DoubleRow

    nc = bacc.Bacc(None, target_bir_lowering=False, debug=False)

    with tile.TileContext(nc) as tc:
        with (
            tc.tile_pool(name="dram", bufs=1, space="DRAM") as dram,
            tc.tile_pool(name="wres", bufs=1) as wres,
            tc.tile_pool(name="einp", bufs=4) as einp,
            tc.tile_pool(name="actp", bufs=14) as actp,
            tc.tile_pool(name="h2p", bufs=h2_bufs) as h2p,
            tc.tile_pool(name="smal", bufs=8) as smal,
            tc.tile_pool(name="aggp", bufs=8) as aggp,
            tc.tile_pool(name="bigps", bufs=6, space="PSUM") as bigps,
            tc.tile_pool(name="segps", bufs=1, space="PSUM") as segps,
        ):
            # ---- DRAM I/O -------------------------------------------------
            d_ein = dram.tile([P, EPAD], f16, kind="ExternalInput", name="ein")
            d_relw = dram.tile([P, NWIN * T], f32, kind="ExternalInput", name="relw")
            d_invc = dram.tile([P, T], f32, kind="ExternalInput", name="invc")
            d_xT2 = dram.tile([P, NPAD], f16, kind="ExternalInput", name="xT2")
            d_iota = dram.tile([P, WN], f32, kind="ExternalInput", name="iota")
            d_n1b2bc = dram.tile([P, N_HID], f32, kind="ExternalInput", name="n1b2bc")

            wspec16 = dict(eW1p=[P, E_HID], n1W1a=[P, N_HID], n2W1x=[P, N_HID],
                           n2W1agg=[N_HID, N_HID], n2W2=[N_HID, 1])
            d_w = {k: dram.tile(s, f16, kind="ExternalInput", name=k)
                   for k, s in wspec16.items()}
            for k in ("eW2", "eW3", "eW4"):
                d_w[k] = dram.tile([P, 8192], f8, kind="ExternalInput", name=k)
            d_w["wcombo"] = dram.tile([P, 4096], f8, kind="ExternalInput", name="wcombo")
            d_w["n1W2"] = dram.tile([P, 2048], f8, kind="ExternalInput", name="n1W2")
            bspec = dict(eb2r64=[P, 8], eb3r=[P, 8], eb4r64=[P, 8], n2b2r=[1, 1])
            d_b = {k: dram.tile(s, f32, kind="ExternalInput", name=k)
                   for k, s in bspec.items()}
            d_z = dram.tile([1, NPAD], f32, kind="ExternalOutput", name="zout")

            names = dict(ein=d_ein.name, relw=d_relw.name, invc=d_invc.name,
                         xT2=d_xT2.name, iota=d_iota.name, n1b2bc=d_n1b2bc.name,
                         zout=d_z.name)
            names.update({k: v.name for k, v in d_w.items()})
            names.update({k: v.name for k, v in d_b.items()})

            # ---- resident loads (spread across queues; L1 needs first) ----
            W1 = wres.tile([P, E_HID], f16, name="w_eW1p")
            nc.scalar.dma_start(out=W1[:], in_=d_w["eW1p"][:])
            B = {}
            for k, s in bspec.items():
                t = wres.tile(s, f32, name=f"b_{k}")
                nc.scalar.dma_start(out=t[:], in_=d_b[k][:])
                B[k] = t
            iota = wres.tile([P, WN], f32, name="iota_sb")
            nc.scalar.dma_start(out=iota[:], in_=d_iota[:])
            invc = wres.tile([P, T], f32, name="invc_sb")
            nc.scalar.dma_start(out=invc[:], in_=d_invc[:])
            n1b2bc = wres.tile([P, N_HID], f32, name="n1b2bc_sb")
            nc.scalar.dma_start(out=n1b2bc[:], in_=d_n1b2bc[:])

            def load_wdr(eng, name, npair, width):
                ts = []
                for q in range(npair):
                    t = wres.tile([P, 2, width], f8, name=f"w_{name}_{q}")
                    eng.dma_start(out=t[:, :, :],
                                  in_=d_w[name][:, q * 2 * width:(q + 1) * 2 * width])
                    ts.append(t)
                return ts

            W2 = load_wdr(nc.scalar, "eW2", 4, E_HID)
            W3 = load_wdr(nc.scalar, "eW3", 4, E_HID)
            W4 = load_wdr(nc.gpsimd, "eW4", 4, E_HID)
            Wc = load_wdr(nc.gpsimd, "wcombo", 4, N_HID)
            W7 = load_wdr(nc.gpsimd, "n1W2", 2, N_HID)
            W6a = wres.tile([P, N_HID], f16, name="w_n1W1a")
            nc.gpsimd.dma_start(out=W6a[:], in_=d_w["n1W1a"][:])
            W8x = wres.tile([P, N_HID], f16, name="w_n2W1x")
            nc.gpsimd.dma_start(out=W8x[:], in_=d_w["n2W1x"][:])
            W8a = []
            for k in range(4):
                t = wres.tile([P, N_HID], f16, name=f"w_n2W1agg_{k}")
                nc.gpsimd.dma_start(out=t[:], in_=d_w["n2W1agg"][k * P:(k + 1) * P, :])
                W8a.append(t)
            W9 = []
            for k in range(4):
                t = wres.tile([P, 1], f16, name=f"w_n2W2_{k}")
                nc.gpsimd.dma_start(out=t[:], in_=d_w["n2W2"][k * P:(k + 1) * P, :])
                W9.append(t)
            relw = wres.tile([P, NWIN * T], f32, name="relw_sb")
            nc.gpsimd.dma_start(out=relw[:], in_=d_relw[:])
            xT2 = wres.tile([P, NPAD], f16, name="xT2_sb")
            nc.gpsimd.dma_start(out=xT2[:], in_=d_xT2[:])

            # h2 pair tiles by pair index (edge tiles 2k, 2k+1)
            h2_pairs = {}

            def dr_layer(pin, Wp, quant, tag):
                """fp8 DoubleRow layer; quant(out_plane, psum, m) applies the
                bias+relu+store for m-tile m."""
                outs = []
                pt = None
                for m in range(8):
                    ps = bigps.tile([P, 512], mybir.dt.float32, name="ps_big")
                    for q in range(len(Wp)):
                        nc.tensor.matmul(
                            out=ps[:], lhsT=Wp[q][:, :, m * P:(m + 1) * P],
                            rhs=pin[q][:, :, :], start=(q == 0),
                            stop=(q == len(Wp) - 1), perf_mode=DR)
                    if m % 2 == 0:
                        pt = actp.tile([P, 2, 512], f8, name=tag)
                        outs.append(pt)
                    quant(pt[:, m % 2, :], ps, m)
                return outs

            def q_dve64(btile):
                def q(o, ps, m):
                    nc.vector.tensor_scalar(
                        out=o, in0=ps[:], scalar1=btile[:, m:m + 1], scalar2=0.0,
                        op0=ADD, op1=MAX)
                return q

            def q_act(btile, scale):
                def q(o, ps, m):
                    nc.scalar.activation(o, ps[:], RELU,
                                         bias=btile[:, m:m + 1], scale=scale)
                return q

            def emit_block(b):
                e0 = b * 1024
                ein_t = einp.tile([P, 1024], f16, name="ein_t")
                nc.sync.dma_start(out=ein_t[:], in_=d_ein[:, e0:e0 + 1024])
                for h in range(2):
                    einh = ein_t[:, h * 512:(h + 1) * 512]
                    # L1 -> a1 (x1) on ACT; bias rides the ones-row of ein
                    a1p = []
                    pt = None
                    for m in range(8):
                        ps = bigps.tile([P, 512], mybir.dt.float32, name="ps_big")
                        nc.tensor.matmul(out=ps[:], lhsT=W1[:, m * P:(m + 1) * P],
                                         rhs=einh, start=True, stop=True)
                        if m % 2 == 0:
                            pt = actp.tile([P, 2, 512], f8, name="pairA")
                            a1p.append(pt)
                        nc.scalar.activation(pt[:, m % 2, :], ps[:], RELU)
                    # L2 -> a2 (x64) on DVE
                    a2p = dr_layer(a1p, W2, q_dve64(B["eb2r64"]), "pairB")
                    # L3 -> a3 (x1) on ACT
                    a3p = dr_layer(a2p, W3, q_act(B["eb3r"], 1.0 / 4096), "pairA")
                    # L4 -> a4 (x64) on DVE
                    a4p = dr_layer(a3p, W4, q_dve64(B["eb4r64"]), "pairB")
                    # combo (eW5 @ n1W1 fused) + fp16 x-part -> h1 (x64) on ACT
                    h1p = []
                    pt = None
                    for m in range(4):
                        ps = bigps.tile([P, 512], mybir.dt.float32, name="ps_big")
                        nc.tensor.matmul(out=ps[:], lhsT=W6a[:, m * P:(m + 1) * P],
                                         rhs=einh, start=True, stop=False)
                        for q in range(4):
                            nc.tensor.matmul(
                                out=ps[:], lhsT=Wc[q][:, :, m * P:(m + 1) * P],
                                rhs=a4p[q][:, :, :], start=False, stop=(q == 3),
                                perf_mode=DR)
                        if m % 2 == 0:
                            pt = actp.tile([P, 2, 512], f8, name="pairC")
                            h1p.append(pt)
                        nc.scalar.activation(pt[:, m % 2, :], ps[:], RELU,
                                             scale=1.0 / 64)
                    # n1 L2: edge-major h2 per 128-edge subtile -> fp8 pairs
                    for s in range(4):
                        t_glob = b * 8 + h * 4 + s
                        ps = bigps.tile([P, 512], mybir.dt.float32, name="ps_big")
                        for q in range(2):
                            nc.tensor.matmul(
                                out=ps[:], lhsT=h1p[q][:, :, s * P:(s + 1) * P],
                                rhs=W7[q][:, :, :], start=(q == 0), stop=(q == 1),
                                perf_mode=DR)
                        tmp = smal.tile([P, 512], f16, name="tmp16")
                        nc.vector.tensor_tensor(out=tmp[:], in0=ps[:],
                                                in1=n1b2bc[:], op=ADD)
                        k, j = t_glob // 2, t_glob % 2
                        if j == 0:
                            h2_pairs[k] = h2p.tile([P, 2, N_HID], f8, name="h2t")
                        nc.vector.tensor_scalar(
                            out=h2_pairs[k][:, j, :], in0=tmp[:],
                            scalar1=invc[:, t_glob:t_glob + 1], scalar2=0.0,
                            op0=MULT, op1=MAX)

            def emit_window(w):
                klo, khi = win_pairs[w]
                seg = segps.tile([P, 4, WN], mybir.dt.float32, name="segps_t")
                pl = list(range(klo, khi + 1))
                for ki_, k in enumerate(pl):
                    memb = smal.tile([P, 2, WN], f8, name="memb")
                    for j in range(2):
                        t = 2 * k + j
                        nc.vector.tensor_scalar(
                            out=memb[:, j, :], in0=iota[:],
                            scalar1=relw[:, w * T + t:w * T + t + 1], scalar2=None,
                            op0=ISEQ)
                    for fc in range(4):
                        nc.tensor.matmul(
                            out=seg[:, fc, :],
                            lhsT=h2_pairs[k][:, :, fc * P:(fc + 1) * P],
                            rhs=memb[:, :, :], start=(ki_ == 0),
                            stop=(ki_ == len(pl) - 1), perf_mode=DR)
                agg = []
                for fc in range(4):
                    at = aggp.tile([P, WN], f16, name="aggt")
                    nc.scalar.activation(at[:], seg[:, fc, :], COPY)
                    agg.append(at)
                # ---- node MLP2 on this 256-node window ----
                n0 = w * WN
                xin = [xT2[:, n0:n0 + WN]] + [a[:] for a in agg]
                Win = [W8x] + W8a
                z1 = []
                for m in range(4):
                    ps = bigps.tile([P, 512], mybir.dt.float32, name="ps_big")
                    pss = ps[:, :WN]
                    for ki in range(5):
                        nc.tensor.matmul(out=pss, lhsT=Win[ki][:, m * P:(m + 1) * P],
                                         rhs=xin[ki], start=(ki == 0), stop=(ki == 4))
                    zt = smal.tile([P, WN], f16, name="z1t")
                    nc.scalar.activation(zt[:], pss, RELU)
                    z1.append(zt)
                ps = bigps.tile([P, 512], mybir.dt.float32, name="ps_big")
                pss = ps[:1, :WN]
                for ki in range(4):
                    nc.tensor.matmul(out=pss, lhsT=W9[ki][:], rhs=z1[ki][:],
                                     start=(ki == 0), stop=(ki == 3))
                zo = smal.tile([1, WN], f32, name="zot")
                nc.vector.tensor_scalar(out=zo[:], in0=pss,
                                        scalar1=B["n2b2r"][0:1, 0:1], scalar2=None,
                                        op0=ADD)
                nc.sync.dma_start(out=d_z[:, n0:n0 + WN], in_=zo[:])

            # window w ready once tile win_tiles[w][1] is produced
            ready = {}
            for w in range(NWIN):
                b_ready = min(NB - 1, win_tiles[w][1] // 8)
                ready.setdefault(b_ready, []).append(w)
            for b in range(NB):
                emit_block(b)
                for w in ready.get(b, []):
                    emit_window(w)

    nc.compile()
    return nc, names


def kernel(x, edge_attr, u, edge_index, batch, Wsel, bsel,
           eW1, eb1, eW2, eb2, eW3, eb3, eW4, eb4, eW5, eb5,
           n1W1, n1b1, n1W2, n1b2, n2W1, n2b1, n2W2, n2b2):
    f32 = np.float32
    f16 = np.float16
    x = np.asarray(x, f32)
    edge_attr = np.asarray(edge_attr, f32)
    u = np.asarray(u, f32)
    edge_index = np.asarray(edge_index)
    batch = np.asarray(batch)
    ws = {k: np.asarray(v, f32) for k, v in dict(
        Wsel=Wsel, bsel=bsel, eW1=eW1, eb1=eb1, eW2=eW2, eb2=eb2, eW3=eW3,
        eb3=eb3, eW4=eW4, eb4=eb4, eW5=eW5, eb5=eb5, n1W1=n1W1, n1b1=n1b1,
        n1W2=n1W2, n1b2=n1b2, n2W1=n2W1, n2b1=n2b1, n2W2=n2W2, n2b2=n2b2).items()}

    # ---------------- host math (index plumbing + tiny matmuls) ------------
    u2 = (u @ ws["Wsel"] + ws["bsel"]).astype(f32)          # [64, 256]
    row = np.asarray(edge_index[0], np.int64)
    col = np.asarray(edge_index[1], np.int64)
    order = np.argsort(row, kind="stable")
    row_s, col_s = row[order], col[order]
    g_s = np.asarray(batch[row_s], np.int64)
    ea_s = edge_attr[order, 0]
    cnt = np.bincount(row, minlength=N_NODES).astype(f32)
    invc_node = (1.0 / np.maximum(cnt, 1.0)).astype(f32)

    bounds = np.searchsorted(row_s, np.arange(0, N_NODES + 1, NPN))
    e_cnt = np.diff(bounds)
    EPAD = int(-(-int(e_cnt.max()) // 1024) * 1024)
    T = EPAD // P

    # per-window tile ranges (shared across cores) for the static program
    tlo = np.full(NWIN, T - 1, np.int64)
    thi = np.zeros(NWIN, np.int64)
    core_dat = []
    for c in range(NC):
        lo, hi = bounds[c], bounds[c + 1]
        n = hi - lo
        rel = np.full(EPAD, 1e6, f32)
        rel[:n] = (row_s[lo:hi] - NPN * c).astype(f32)
        w_of_edge = np.floor_divide(rel[:n].astype(np.int64), WN)
        for w in range(NWIN):
            idx = np.nonzero(w_of_edge == w)[0]
            if idx.size:
                tlo[w] = min(tlo[w], idx[0] // P)
                thi[w] = max(thi[w], idx[-1] // P)
        core_dat.append((lo, hi, n, rel))
    win_tiles = [(int(tlo[w]), int(max(tlo[w], thi[w]))) for w in range(NWIN)]
    h2_bufs = max(th // 2 - tl // 2 + 1 for tl, th in win_tiles) + 10

    # ---------------- shared weight packing ---------------------------------
    G64 = u2 @ ws["eW1"][19:275]                      # [64, 1024]
    eW1p = np.zeros((P, E_HID), f32)
    eW1p[0:9] = ws["eW1"][9:18]       # x[col] (dest)
    eW1p[9:18] = ws["eW1"][0:9]       # x[row] (src)
    eW1p[18] = ws["eW1"][18]          # edge_attr
    eW1p[19:83] = G64                 # one-hot graph -> u2 @ eW1
    eW1p[83] = ws["eb1"]              # ones-row bias

    Wc_mat = ws["eW5"] @ ws["n1W1"][9:521]            # [1024, 512]
    b_combo = ws["eb5"] @ ws["n1W1"][9:521] + ws["n1b1"]
    n1W1a = np.zeros((P, N_HID), f32)
    n1W1a[0:9] = ws["n1W1"][0:9] * 4096.0
    n1W1a[83] = b_combo * 4096.0

    H64 = u2 @ ws["n2W1"][521:777]                    # [64, 512]
    n2W1x = np.zeros((P, N_HID), f32)
    n2W1x[0:9] = ws["n2W1"][0:9]
    n2W1x[9:73] = H64
    n2W1x[73] = ws["n2b1"]

    def br(b, nm):   # bias [nm*128] -> [128, nm]
        return np.ascontiguousarray(b.reshape(nm, P).T).astype(f32)

    import ml_dtypes
    fp8 = ml_dtypes.float8_e4m3

    def packdr(W):   # [K, M] -> [128, (K//128)*M] fp8, x64, (q,j,m) free order
        K, M = W.shape
        Wp = (W * 64.0).reshape(K // 256, 2, P, M)
        return np.ascontiguousarray(
            np.transpose(Wp, (2, 0, 1, 3)).reshape(P, (K // P) * M)).astype(fp8)

    shared = dict(
        eW1p=eW1p.astype(f16), eW2=packdr(ws["eW2"]),
        eW3=packdr(ws["eW3"]), eW4=packdr(ws["eW4"]),
        wcombo=packdr(Wc_mat), n1W2=packdr(ws["n1W2"]),
        n1W1a=n1W1a.astype(f16), n2W1x=n2W1x.astype(f16),
        n2W1agg=(ws["n2W1"][9:521] / 64.0).astype(f16),
        n2W2=ws["n2W2"].astype(f16),
        eb2r64=br(ws["eb2"] * 64.0, 8), eb3r=br(ws["eb3"], 8),
        eb4r64=br(ws["eb4"] * 64.0, 8),
        n2b2r=ws["n2b2"].reshape(1, 1).astype(f32),
        iota=np.tile(np.arange(WN, dtype=f32), (P, 1)),
        n1b2bc=np.tile(ws["n1b2"].astype(f32) * 4096.0, (P, 1)),
    )

    # ---------------- per-core input packing --------------------------------
    in_maps = []
    for c in range(NC):
        lo, hi, n, rel = core_dat[c]
        ein = np.zeros((P, EPAD), f32)
        ein[0:9, :n] = x[col_s[lo:hi]].T
        ein[9:18, :n] = x[row_s[lo:hi]].T
        ein[18, :n] = ea_s[lo:hi]
        ein[19 + g_s[lo:hi], np.arange(n)] = 1.0
        ein[83, :n] = 1.0
        relw = np.empty((P, NWIN * T), f32)
        for w in range(NWIN):
            relw[:, w * T:(w + 1) * T] = _pack_cols(rel - float(WN) * w, T)
        invc_e = np.ones(EPAD, f32)
        invc_e[:n] = invc_node[row_s[lo:hi]]
        invc_e *= 1.0 / 64.0
        xT2 = np.zeros((P, NPAD), f32)
        xT2[0:9, :NPN] = x[NPN * c:NPN * (c + 1)].T
        bc = np.asarray(batch[NPN * c:NPN * (c + 1)], np.int64)
        xT2[9 + bc, np.arange(NPN)] = 1.0
        xT2[73, :NPN] = 1.0
        im = dict(shared)
        im.update(ein=ein.astype(f16), relw=relw,
                  invc=_pack_cols(invc_e, T), xT2=xT2.astype(f16))
        in_maps.append(im)

    # ---------------- build + run ------------------------------------------
    key = (EPAD, tuple(win_tiles))
    if key not in _CACHE:
        _CACHE[key] = _build_module(EPAD, win_tiles, h2_bufs)
    nc, names = _CACHE[key]

    from concourse import bass_utils
    trace = bool(int(os.environ.get("KERNEL_TRACE", "0")))
    if trace:
        try:
            import types
            import antenv
            if not hasattr(antenv, "axon_hooks"):
                mod = types.ModuleType("antenv.axon_hooks")
                mod._hook = None
                mod.set_axon_ntff_profile_hook = lambda h: setattr(mod, "_hook", h)
                mod.get_axon_ntff_profile_hook = lambda: mod._hook
                sys.modules["antenv.axon_hooks"] = mod
                antenv.axon_hooks = mod
                from trn_agent_boot.trn_boot import _ntff_profile_via_ctypes
                mod._hook = _ntff_profile_via_ctypes("/opt/axon/libaxon_pjrt.so")
        except Exception as e:  # profiling is best-effort
            print("ntff hook shim failed:", e)
            trace = False
    real_maps = [{names[k]: v for k, v in im.items()} for im in in_maps]
    res = bass_utils.run_bass_kernel_spmd(
        nc, real_maps, core_ids=list(range(NC)), trace=trace)
    if trace and res.exec_time_ns is not None:
        print(f"HW exec time: {res.exec_time_ns} ns")
        if res.instructions_and_trace:
            print("trace:", res.instructions_and_trace[1])

    out = np.empty(N_NODES, f32)
    for c in range(NC):
        out[NPN * c:NPN * (c + 1)] = res.results[c][names["zout"]][0, :NPN]
    return out


# revision 6
# speedup vs baseline: 1.2118x; 1.0027x over previous
"""OGRENet GNN message-passing kernel for 8 Trainium2 NeuronCores.

Strategy
--------
Host (numpy, cheap index plumbing only):
  * u2 = u @ Wsel + bsel (64x256), G64 = u2 @ eW1[19:275] (64x1024),
    H64 = u2 @ n2W1[521:777] (64x512): the per-edge/node u2 gather becomes a
    64-wide one-hot riding inside the K=128 first-layer matmul.
  * W_combo = eW5 @ n1W1[9:521]: e5 is never materialized (it is only ever
    consumed linearly by node MLP1), fusing two layers into one.
  * sort edges by destination node (`row`), split into 8 contiguous chunks at
    node boundaries -> each core owns a contiguous node range and ALL edges
    that scatter into it => no cross-core reduction.
  * pack per-core feature-major edge inputs ein = [x[col]; x[row]; e_attr;
    onehot(g); 1] (fp16, 128 rows total), per-edge 1/count scales, and
    window-relative row ids.

Device (per core, identical program, different data):
  * edge MLP layers as fp8 DoubleRow matmuls (157 TF/s), fp32 PSUM. Biases and
    relu are applied in a scale-staged way so the quantize ops spread across
    ACT/DVE/GpSimd and the tensor engine never waits:
      a1 = relu(ps)            x1   ACT   (bias via ones-row in ein)
      a2 = max(ps + 64 b2, 0)  x64  DVE   (2-op tensor_scalar)
      a3 = relu(ps/4096 + b3)  x1   ACT
      a4 = max(ps + 64 b4, 0)  x64  DVE
      h1 = relu(ps/64)         x64  ACT   (bias via ones-row in x-part)
      h2 = max((ps+4096 b) * invc/64, 0) fp8 x64  GpSimd add + DVE mul/relu
  * segment-sum as matmul with fp8 DoubleRow: membership pairs M[e,n]
    (is_equal on DVE) contract TWO 128-edge tiles per pass into a
    [feat, 256-node] PSUM window.
  * node MLP2 consumes agg feature-major; u2[batch] + bias enter through
    one-hot/ones rows of the x input tile; z -> DRAM.
"""

import os
import sys

import numpy as np

sys.path.insert(0, "/opt/trn_rl_repo")

N_NODES = 20000
N_GRAPHS = 64
E_HID = 1024
N_HID = 512
NC = 8
NPN = N_NODES // NC          # nodes per core (2500)
NPAD = 2560                  # padded nodes per core
WN = 256                     # nodes per segment window
NWIN = NPAD // WN            # 10
P = 128

_CACHE = {}


def _pack_cols(v, T):
    """[T*128] -> [128, T] with col t = v[t*128:(t+1)*128]."""
    return np.ascontiguousarray(v.reshape(T, P).T)


def _build_module(EPAD, win_tiles, h2_bufs):
    """Build the per-core Bass program. win_tiles[w] = (tlo, thi) inclusive."""
    from concourse import bacc, mybir, tile

    T = EPAD // P           # 128-edge tiles
    NB = EPAD // 1024       # 1024-edge blocks
    win_pairs = [(tl // 2, th // 2) for tl, th in win_tiles]
    f16 = mybir.dt.float16
    f32 = mybir.dt.float32
    f8 = mybir.dt.float8e4
    RELU = mybir.ActivationFunctionType.Relu
    COPY = mybir.ActivationFunctionType.Copy
    ADD = mybir.AluOpType.add
    MULT = mybir.AluOpType.mult
    MAX = mybir.AluOpType.max
    ISEQ = mybir.AluOpType.is_equal
    DR = mybir.MatmulPerfMode.DoubleRow

    nc = bacc.Bacc(None, target_bir_lowering=False, debug=False)

    with tile.TileContext(nc) as tc:
        with (
            tc.tile_pool(name="dram", bufs=1, space="DRAM") as dram,
            tc.tile_pool(name="wres", bufs=1) as wres,
            tc.tile_pool(name="einp", bufs=4) as einp,
            tc.tile_pool(name="actp", bufs=14) as actp,
            tc.tile_pool(name="h2p", bufs=h2_bufs) as h2p,
            tc.tile_pool(name="smal", bufs=8) as smal,
            tc.tile_pool(name="aggp", bufs=8) as aggp,
            tc.tile_pool(name="bigps", bufs=6, space="PSUM") as bigps,
            tc.tile_pool(name="segps", bufs=1, space="PSUM") as segps,
        ):
            # ---- DRAM I/O -------------------------------------------------
            d_ein = dram.tile([P, EPAD], f16, kind="ExternalInput", name="ein")
            d_relw = dram.tile([P, NWIN * T], f32, kind="ExternalInput", name="relw")
            d_invc = dram.tile([P, T], f32, kind="ExternalInput", name="invc")
            d_xT2 = dram.tile([P, NPAD], f16, kind="ExternalInput", name="xT2")
            d_iota = dram.tile([P, WN], f32, kind="ExternalInput", name="iota")
            d_n1b2bc = dram.tile([P, N_HID], f32, kind="ExternalInput", name="n1b2bc")

            wspec16 = dict(eW1p=[P, E_HID], n1W1a=[P, N_HID], n2W1x=[P, N_HID],
                           n2W1agg=[N_HID, N_HID], n2W2=[N_HID, 1])
            d_w = {k: dram.tile(s, f16, kind="ExternalInput", name=k)
                   for k, s in wspec16.items()}
            for k in ("eW2", "eW3", "eW4"):
                d_w[k] = dram.tile([P, 8192], f8, kind="ExternalInput", name=k)
            d_w["wcombo"] = dram.tile([P, 4096], f8, kind="ExternalInput", name="wcombo")
            d_w["n1W2"] = dram.tile([P, 2048], f8, kind="ExternalInput", name="n1W2")
            bspec = dict(eb2r64=[P, 8], eb3r=[P, 8], eb4r64=[P, 8], n2b2r=[1, 1])
            d_b = {k: dram.tile(s, f32, kind="ExternalInput", name=k)
                   for k, s in bspec.items()}
            d_z = dram.tile([1, NPAD], f32, kind="ExternalOutput", name="zout")

            names = dict(ein=d_ein.name, relw=d_relw.name, invc=d_invc.name,
                         xT2=d_xT2.name, iota=d_iota.name, n1b2bc=d_n1b2bc.name,
                         zout=d_z.name)
            names.update({k: v.name for k, v in d_w.items()})
            names.update({k: v.name for k, v in d_b.items()})

            # ---- resident loads --------------------------------------------
            # SP carries only the streamed ein blocks; weights alternate
            # between the ACT and Pool queues in the order compute needs
            # them, so each layer's pairs transfer on two queues in parallel.
            qs = [nc.scalar, nc.gpsimd]

            W1 = wres.tile([P, E_HID], f16, name="w_eW1p")
            nc.scalar.dma_start(out=W1[:], in_=d_w["eW1p"][:])

            def load_wdr(name, npair, width):
                ts = []
                for q in range(npair):
                    t = wres.tile([P, 2, width], f8, name=f"w_{name}_{q}")
                    qs[q % 2].dma_start(
                        out=t[:, :, :],
                        in_=d_w[name][:, q * 2 * width:(q + 1) * 2 * width])
                    ts.append(t)
                return ts

            W2 = load_wdr("eW2", 4, E_HID)
            W3 = load_wdr("eW3", 4, E_HID)
            W4 = load_wdr("eW4", 4, E_HID)
            Wc = load_wdr("wcombo", 4, N_HID)
            W7 = load_wdr("n1W2", 2, N_HID)
            W6a = wres.tile([P, N_HID], f16, name="w_n1W1a")
            nc.scalar.dma_start(out=W6a[:], in_=d_w["n1W1a"][:])
            B = {}
            for k, s in bspec.items():
                t = wres.tile(s, f32, name=f"b_{k}")
                nc.scalar.dma_start(out=t[:], in_=d_b[k][:])
                B[k] = t
            iota = wres.tile([P, WN], f32, name="iota_sb")
            nc.scalar.dma_start(out=iota[:], in_=d_iota[:])
            invc = wres.tile([P, T], f32, name="invc_sb")
            nc.scalar.dma_start(out=invc[:], in_=d_invc[:])
            n1b2bc = wres.tile([P, N_HID], f32, name="n1b2bc_sb")
            nc.gpsimd.dma_start(out=n1b2bc[:], in_=d_n1b2bc[:])
            W8x = wres.tile([P, N_HID], f16, name="w_n2W1x")
            nc.gpsimd.dma_start(out=W8x[:], in_=d_w["n2W1x"][:])
            W8a = []
            for k in range(4):
                t = wres.tile([P, N_HID], f16, name=f"w_n2W1agg_{k}")
                nc.gpsimd.dma_start(out=t[:], in_=d_w["n2W1agg"][k * P:(k + 1) * P, :])
                W8a.append(t)
            W9 = []
            for k in range(4):
                t = wres.tile([P, 1], f16, name=f"w_n2W2_{k}")
                nc.gpsimd.dma_start(out=t[:], in_=d_w["n2W2"][k * P:(k + 1) * P, :])
                W9.append(t)
            relw = wres.tile([P, NWIN * T], f32, name="relw_sb")
            nc.gpsimd.dma_start(out=relw[:], in_=d_relw[:])
            xT2 = wres.tile([P, NPAD], f16, name="xT2_sb")
            nc.gpsimd.dma_start(out=xT2[:], in_=d_xT2[:])

            # h2 pair tiles by pair index (edge tiles 2k, 2k+1)
            h2_pairs = {}

            def dr_layer(pin, Wp, quant, tag):
                """fp8 DoubleRow layer; quant(out_plane, psum, m) applies the
                bias+relu+store for m-tile m."""
                outs = []
                pt = None
                for m in range(8):
                    ps = bigps.tile([P, 512], mybir.dt.float32, name="ps_big")
                    for q in range(len(Wp)):
                        nc.tensor.matmul(
                            out=ps[:], lhsT=Wp[q][:, :, m * P:(m + 1) * P],
                            rhs=pin[q][:, :, :], start=(q == 0),
                            stop=(q == len(Wp) - 1), perf_mode=DR)
                    if m % 2 == 0:
                        pt = actp.tile([P, 2, 512], f8, name=tag)
                        outs.append(pt)
                    quant(pt[:, m % 2, :], ps, m)
                return outs

            def q_dve64(btile):
                def q(o, ps, m):
                    nc.vector.tensor_scalar(
                        out=o, in0=ps[:], scalar1=btile[:, m:m + 1], scalar2=0.0,
                        op0=ADD, op1=MAX)
                return q

            def q_act(btile, scale):
                def q(o, ps, m):
                    nc.scalar.activation(o, ps[:], RELU,
                                         bias=btile[:, m:m + 1], scale=scale)
                return q

            def emit_block(b):
                e0 = b * 1024
                ein_t = einp.tile([P, 1024], f16, name="ein_t")
                nc.sync.dma_start(out=ein_t[:], in_=d_ein[:, e0:e0 + 1024])
                for h in range(2):
                    einh = ein_t[:, h * 512:(h + 1) * 512]
                    # L1 -> a1 (x1) on ACT; bias rides the ones-row of ein
                    a1p = []
                    pt = None
                    for m in range(8):
                        ps = bigps.tile([P, 512], mybir.dt.float32, name="ps_big")
                        nc.tensor.matmul(out=ps[:], lhsT=W1[:, m * P:(m + 1) * P],
                                         rhs=einh, start=True, stop=True)
                        if m % 2 == 0:
                            pt = actp.tile([P, 2, 512], f8, name="pairA")
                            a1p.append(pt)
                        nc.scalar.activation(pt[:, m % 2, :], ps[:], RELU)
                    # L2 -> a2 (x64) on DVE
                    a2p = dr_layer(a1p, W2, q_dve64(B["eb2r64"]), "pairB")
                    # L3 -> a3 (x1) on ACT
                    a3p = dr_layer(a2p, W3, q_act(B["eb3r"], 1.0 / 4096), "pairA")
                    # L4 -> a4 (x64) on DVE
                    a4p = dr_layer(a3p, W4, q_dve64(B["eb4r64"]), "pairB")
                    # combo (eW5 @ n1W1 fused) + fp16 x-part -> h1 (x64) on ACT
                    h1p = []
                    pt = None
                    for m in range(4):
                        ps = bigps.tile([P, 512], mybir.dt.float32, name="ps_big")
                        nc.tensor.matmul(out=ps[:], lhsT=W6a[:, m * P:(m + 1) * P],
                                         rhs=einh, start=True, stop=False)
                        for q in range(4):
                            nc.tensor.matmul(
                                out=ps[:], lhsT=Wc[q][:, :, m * P:(m + 1) * P],
                                rhs=a4p[q][:, :, :], start=False, stop=(q == 3),
                                perf_mode=DR)
                        if m % 2 == 0:
                            pt = actp.tile([P, 2, 512], f8, name="pairC")
                            h1p.append(pt)
                        nc.scalar.activation(pt[:, m % 2, :], ps[:], RELU,
                                             scale=1.0 / 64)
                    # n1 L2: edge-major h2 per 128-edge subtile -> fp8 pairs
                    for s in range(4):
                        t_glob = b * 8 + h * 4 + s
                        ps = bigps.tile([P, 512], mybir.dt.float32, name="ps_big")
                        for q in range(2):
                            nc.tensor.matmul(
                                out=ps[:], lhsT=h1p[q][:, :, s * P:(s + 1) * P],
                                rhs=W7[q][:, :, :], start=(q == 0), stop=(q == 1),
                                perf_mode=DR)
                        tmp = smal.tile([P, 512], f16, name="tmp16")
                        nc.vector.tensor_tensor(out=tmp[:], in0=ps[:],
                                                in1=n1b2bc[:], op=ADD)
                        k, j = t_glob // 2, t_glob % 2
                        if j == 0:
                            h2_pairs[k] = h2p.tile([P, 2, N_HID], f8, name="h2t")
                        nc.vector.tensor_scalar(
                            out=h2_pairs[k][:, j, :], in0=tmp[:],
                            scalar1=invc[:, t_glob:t_glob + 1], scalar2=0.0,
                            op0=MULT, op1=MAX)

            def emit_window(w):
                klo, khi = win_pairs[w]
                seg = segps.tile([P, 4, WN], mybir.dt.float32, name="segps_t")
                pl = list(range(klo, khi + 1))
                for ki_, k in enumerate(pl):
                    memb = smal.tile([P, 2, WN], f8, name="memb")
                    for j in range(2):
                        t = 2 * k + j
                        nc.vector.tensor_scalar(
                            out=memb[:, j, :], in0=iota[:],
                            scalar1=relw[:, w * T + t:w * T + t + 1], scalar2=None,
                            op0=ISEQ)
                    for fc in range(4):
                        nc.tensor.matmul(
                            out=seg[:, fc, :],
                            lhsT=h2_pairs[k][:, :, fc * P:(fc + 1) * P],
                            rhs=memb[:, :, :], start=(ki_ == 0),
                            stop=(ki_ == len(pl) - 1), perf_mode=DR)
                agg = []
                for fc in range(4):
                    at = aggp.tile([P, WN], f16, name="aggt")
                    nc.scalar.activation(at[:], seg[:, fc, :], COPY)
                    agg.append(at)
                # ---- node MLP2 on this 256-node window ----
                n0 = w * WN
                xin = [xT2[:, n0:n0 + WN]] + [a[:] for a in agg]
                Win = [W8x] + W8a
                z1 = []
                for m in range(4):
                    ps = bigps.tile([P, 512], mybir.dt.float32, name="ps_big")
                    pss = ps[:, :WN]
                    for ki in range(5):
                        nc.tensor.matmul(out=pss, lhsT=Win[ki][:, m * P:(m + 1) * P],
                                         rhs=xin[ki], start=(ki == 0), stop=(ki == 4))
                    zt = smal.tile([P, WN], f16, name="z1t")
                    nc.scalar.activation(zt[:], pss, RELU)
                    z1.append(zt)
                ps = bigps.tile([P, 512], mybir.dt.float32, name="ps_big")
                pss = ps[:1, :WN]
                for ki in range(4):
                    nc.tensor.matmul(out=pss, lhsT=W9[ki][:], rhs=z1[ki][:],
                                     start=(ki == 0), stop=(ki == 3))
                zo = smal.tile([1, WN], f32, name="zot")
                nc.vector.tensor_scalar(out=zo[:], in0=pss,
                                        scalar1=B["n2b2r"][0:1, 0:1], scalar2=None,
                                        op0=ADD)
                nc.sync.dma_start(out=d_z[:, n0:n0 + WN], in_=zo[:])

            # window w ready once tile win_tiles[w][1] is produced
            ready = {}
            for w in range(NWIN):
                b_ready = min(NB - 1, win_tiles[w][1] // 8)
                ready.setdefault(b_ready, []).append(w)
            for b in range(NB):
                emit_block(b)
                for w in ready.get(b, []):
                    emit_window(w)

    nc.compile()
    return nc, names


def kernel(x, edge_attr, u, edge_index, batch, Wsel, bsel,
           eW1, eb1, eW2, eb2, eW3, eb3, eW4, eb4, eW5, eb5,
           n1W1, n1b1, n1W2, n1b2, n2W1, n2b1, n2W2, n2b2):
    f32 = np.float32
    f16 = np.float16
    x = np.asarray(x, f32)
    edge_attr = np.asarray(edge_attr, f32)
    u = np.asarray(u, f32)
    edge_index = np.asarray(edge_index)
    batch = np.asarray(batch)
    ws = {k: np.asarray(v, f32) for k, v in dict(
        Wsel=Wsel, bsel=bsel, eW1=eW1, eb1=eb1, eW2=eW2, eb2=eb2, eW3=eW3,
        eb3=eb3, eW4=eW4, eb4=eb4, eW5=eW5, eb5=eb5, n1W1=n1W1, n1b1=n1b1,
        n1W2=n1W2, n1b2=n1b2, n2W1=n2W1, n2b1=n2b1, n2W2=n2W2, n2b2=n2b2).items()}

    # ---------------- host math (index plumbing + tiny matmuls) ------------
    u2 = (u @ ws["Wsel"] + ws["bsel"]).astype(f32)          # [64, 256]
    row = np.asarray(edge_index[0], np.int64)
    col = np.asarray(edge_index[1], np.int64)
    order = np.argsort(row, kind="stable")
    row_s, col_s = row[order], col[order]
    g_s = np.asarray(batch[row_s], np.int64)
    ea_s = edge_attr[order, 0]
    cnt = np.bincount(row, minlength=N_NODES).astype(f32)
    invc_node = (1.0 / np.maximum(cnt, 1.0)).astype(f32)

    bounds = np.searchsorted(row_s, np.arange(0, N_NODES + 1, NPN))
    e_cnt = np.diff(bounds)
    EPAD = int(-(-int(e_cnt.max()) // 1024) * 1024)
    T = EPAD // P

    # per-window tile ranges (shared across cores) for the static program
    tlo = np.full(NWIN, T - 1, np.int64)
    thi = np.zeros(NWIN, np.int64)
    core_dat = []
    for c in range(NC):
        lo, hi = bounds[c], bounds[c + 1]
        n = hi - lo
        rel = np.full(EPAD, 1e6, f32)
        rel[:n] = (row_s[lo:hi] - NPN * c).astype(f32)
        w_of_edge = np.floor_divide(rel[:n].astype(np.int64), WN)
        for w in range(NWIN):
            idx = np.nonzero(w_of_edge == w)[0]
            if idx.size:
                tlo[w] = min(tlo[w], idx[0] // P)
                thi[w] = max(thi[w], idx[-1] // P)
        core_dat.append((lo, hi, n, rel))
    win_tiles = [(int(tlo[w]), int(max(tlo[w], thi[w]))) for w in range(NWIN)]
    h2_bufs = max(th // 2 - tl // 2 + 1 for tl, th in win_tiles) + 10

    # ---------------- shared weight packing ---------------------------------
    G64 = u2 @ ws["eW1"][19:275]                      # [64, 1024]
    eW1p = np.zeros((P, E_HID), f32)
    eW1p[0:9] = ws["eW1"][9:18]       # x[col] (dest)
    eW1p[9:18] = ws["eW1"][0:9]       # x[row] (src)
    eW1p[18] = ws["eW1"][18]          # edge_attr
    eW1p[19:83] = G64                 # one-hot graph -> u2 @ eW1
    eW1p[83] = ws["eb1"]              # ones-row bias

    Wc_mat = ws["eW5"] @ ws["n1W1"][9:521]            # [1024, 512]
    b_combo = ws["eb5"] @ ws["n1W1"][9:521] + ws["n1b1"]
    n1W1a = np.zeros((P, N_HID), f32)
    n1W1a[0:9] = ws["n1W1"][0:9] * 4096.0
    n1W1a[83] = b_combo * 4096.0

    H64 = u2 @ ws["n2W1"][521:777]                    # [64, 512]
    n2W1x = np.zeros((P, N_HID), f32)
    n2W1x[0:9] = ws["n2W1"][0:9]
    n2W1x[9:73] = H64
    n2W1x[73] = ws["n2b1"]

    def br(b, nm):   # bias [nm*128] -> [128, nm]
        return np.ascontiguousarray(b.reshape(nm, P).T).astype(f32)

    import ml_dtypes
    fp8 = ml_dtypes.float8_e4m3

    def packdr(W):   # [K, M] -> [128, (K//128)*M] fp8, x64, (q,j,m) free order
        K, M = W.shape
        Wp = (W * 64.0).reshape(K // 256, 2, P, M)
        return np.ascontiguousarray(
            np.transpose(Wp, (2, 0, 1, 3)).reshape(P, (K // P) * M)).astype(fp8)

    shared = dict(
        eW1p=eW1p.astype(f16), eW2=packdr(ws["eW2"]),
        eW3=packdr(ws["eW3"]), eW4=packdr(ws["eW4"]),
        wcombo=packdr(Wc_mat), n1W2=packdr(ws["n1W2"]),
        n1W1a=n1W1a.astype(f16), n2W1x=n2W1x.astype(f16),
        n2W1agg=(ws["n2W1"][9:521] / 64.0).astype(f16),
        n2W2=ws["n2W2"].astype(f16),
        eb2r64=br(ws["eb2"] * 64.0, 8), eb3r=br(ws["eb3"], 8),
        eb4r64=br(ws["eb4"] * 64.0, 8),
        n2b2r=ws["n2b2"].reshape(1, 1).astype(f32),
        iota=np.tile(np.arange(WN, dtype=f32), (P, 1)),
        n1b2bc=np.tile(ws["n1b2"].astype(f32) * 4096.0, (P, 1)),
    )

    # ---------------- per-core input packing --------------------------------
    in_maps = []
    for c in range(NC):
        lo, hi, n, rel = core_dat[c]
        ein = np.zeros((P, EPAD), f32)
        ein[0:9, :n] = x[col_s[lo:hi]].T
        ein[9:18, :n] = x[row_s[lo:hi]].T
        ein[18, :n] = ea_s[lo:hi]
        ein[19 + g_s[lo:hi], np.arange(n)] = 1.0
        ein[83, :n] = 1.0
        relw = np.empty((P, NWIN * T), f32)
        for w in range(NWIN):
            relw[:, w * T:(w + 1) * T] = _pack_cols(rel - float(WN) * w, T)
        invc_e = np.ones(EPAD, f32)
        invc_e[:n] = invc_node[row_s[lo:hi]]
        invc_e *= 1.0 / 64.0
        xT2 = np.zeros((P, NPAD), f32)
        xT2[0:9, :NPN] = x[NPN * c:NPN * (c + 1)].T
        bc = np.asarray(batch[NPN * c:NPN * (c + 1)], np.int64)
        xT2[9 + bc, np.arange(NPN)] = 1.0
        xT2[73, :NPN] = 1.0
        im = dict(shared)
        im.update(ein=ein.astype(f16), relw=relw,
                  invc=_pack_cols(invc_e, T), xT2=xT2.astype(f16))
        in_maps.append(im)

    # ---------------- build + run ------------------------------------------
    key = (EPAD, tuple(win_tiles))
    if key not in _CACHE:
        _CACHE[key] = _build_module(EPAD, win_tiles, h2_bufs)
    nc, names = _CACHE[key]

    from concourse import bass_utils
    trace = bool(int(os.environ.get("KERNEL_TRACE", "0")))
    if trace:
        try:
            import types
            import antenv
            if not hasattr(antenv, "axon_hooks"):
                mod = types.ModuleType("antenv.axon_hooks")
                mod._hook = None
                mod.set_axon_ntff_profile_hook = lambda h: setattr(mod, "_hook", h)
                mod.get_axon_ntff_profile_hook = lambda: mod._hook
                sys.modules["antenv.axon_hooks"] = mod
                antenv.axon_hooks = mod
                from trn_agent_boot.trn_boot import _ntff_profile_via_ctypes
                mod._hook = _ntff_profile_via_ctypes("/opt/axon/libaxon_pjrt.so")
        except Exception as e:  # profiling is best-effort
            print("ntff hook shim failed:", e)
            trace = False
    real_maps = [{names[k]: v for k, v in im.items()} for im in in_maps]
    res = bass_utils.run_bass_kernel_spmd(
        nc, real_maps, core_ids=list(range(NC)), trace=trace)
    if trace and res.exec_time_ns is not None:
        print(f"HW exec time: {res.exec_time_ns} ns")
        if res.instructions_and_trace:
            print("trace:", res.instructions_and_trace[1])

    out = np.empty(N_NODES, f32)
    for c in range(NC):
        out[NPN * c:NPN * (c + 1)] = res.results[c][names["zout"]][0, :NPN]
    return out


# revision 11
# speedup vs baseline: 1.2149x; 1.0025x over previous
"""OGRENet GNN message-passing kernel for 8 Trainium2 NeuronCores.

Strategy
--------
Host (numpy, cheap index plumbing only):
  * u2 = u @ Wsel + bsel (64x256), G64 = u2 @ eW1[19:275] (64x1024),
    H64 = u2 @ n2W1[521:777] (64x512): the per-edge/node u2 gather becomes a
    64-wide one-hot riding inside the K=128 first-layer matmul.
  * W_combo = eW5 @ n1W1[9:521]: e5 is never materialized (it is only ever
    consumed linearly by node MLP1), fusing two layers into one.
  * sort edges by destination node (`row`), split into 8 contiguous chunks at
    node boundaries -> each core owns a contiguous node range and ALL edges
    that scatter into it => no cross-core reduction.
  * pack per-core feature-major edge inputs ein = [x[col]; x[row]; e_attr;
    onehot(g); 1] (fp16, 128 rows total), per-edge 1/count scales, and
    window-relative row ids.

Device (per core, identical program, different data):
  * edge MLP layers as fp8 DoubleRow matmuls (157 TF/s), fp32 PSUM. Biases and
    relu are applied in a scale-staged way so the quantize ops spread across
    ACT/DVE/GpSimd and the tensor engine never waits:
      a1 = relu(ps)            x1   ACT   (bias via ones-row in ein)
      a2 = max(ps + 64 b2, 0)  x64  DVE   (2-op tensor_scalar)
      a3 = relu(ps/4096 + b3)  x1   ACT
      a4 = max(ps + 64 b4, 0)  x64  DVE
      h1 = relu(ps/64)         x64  ACT   (bias via ones-row in x-part)
      h2 = max((ps+4096 b) * invc/64, 0) fp8 x64  GpSimd add + DVE mul/relu
  * segment-sum as matmul with fp8 DoubleRow: membership pairs M[e,n]
    (is_equal on DVE) contract TWO 128-edge tiles per pass into a
    [feat, 256-node] PSUM window.
  * node MLP2 consumes agg feature-major; u2[batch] + bias enter through
    one-hot/ones rows of the x input tile; z -> DRAM.
"""

import os
import sys

import numpy as np

sys.path.insert(0, "/opt/trn_rl_repo")

N_NODES = 20000
N_GRAPHS = 64
E_HID = 1024
N_HID = 512
NC = 8
NPN = N_NODES // NC          # nodes per core (2500)
NPAD = 2560                  # padded nodes per core
WN = 256                     # nodes per segment window
NWIN = NPAD // WN            # 10
P = 128

_CACHE = {}


def _pack_cols(v, T):
    """[T*128] -> [128, T] with col t = v[t*128:(t+1)*128]."""
    return np.ascontiguousarray(v.reshape(T, P).T)


def _build_module(EPAD, win_tiles, h2_bufs):
    """Build the per-core Bass program. win_tiles[w] = (tlo, thi) inclusive."""
    from concourse import bacc, mybir, tile

    T = EPAD // P           # 128-edge tiles
    NB = EPAD // 1024       # 1024-edge blocks
    win_pairs = [(tl // 2, th // 2) for tl, th in win_tiles]
    f16 = mybir.dt.float16
    f32 = mybir.dt.float32
    f8 = mybir.dt.float8e4
    RELU = mybir.ActivationFunctionType.Relu
    COPY = mybir.ActivationFunctionType.Copy
    ADD = mybir.AluOpType.add
    MULT = mybir.AluOpType.mult
    MAX = mybir.AluOpType.max
    ISEQ = mybir.AluOpType.is_equal
    DR = mybir.MatmulPerfMode.DoubleRow

    nc = bacc.Bacc(None, target_bir_lowering=False, debug=False)

    with tile.TileContext(nc) as tc:
        with (
            tc.tile_pool(name="dram", bufs=1, space="DRAM") as dram,
            tc.tile_pool(name="wres", bufs=1) as wres,
            tc.tile_pool(name="einp", bufs=4) as einp,
            tc.tile_pool(name="actp", bufs=14) as actp,
            tc.tile_pool(name="h2p", bufs=h2_bufs) as h2p,
            tc.tile_pool(name="smal", bufs=8) as smal,
            tc.tile_pool(name="aggp", bufs=12) as aggp,
            tc.tile_pool(name="bigps", bufs=6, space="PSUM") as bigps,
            tc.tile_pool(name="segps", bufs=1, space="PSUM") as segps,
        ):
            # ---- DRAM I/O -------------------------------------------------
            d_ein = dram.tile([P, EPAD], f16, kind="ExternalInput", name="ein")
            d_relw = dram.tile([P, NWIN * T], f32, kind="ExternalInput", name="relw")
            d_invc = dram.tile([P, T], f32, kind="ExternalInput", name="invc")
            d_xT2 = dram.tile([P, NPAD], f16, kind="ExternalInput", name="xT2")
            d_iota = dram.tile([P, WN], f32, kind="ExternalInput", name="iota")
            d_n1b2bc = dram.tile([P, N_HID], f32, kind="ExternalInput", name="n1b2bc")

            wspec16 = dict(eW1p=[P, E_HID], n1W1a=[P, N_HID], n2W1x=[P, N_HID],
                           n2W1agg=[N_HID, N_HID], n2W2=[N_HID, 1])
            d_w = {k: dram.tile(s, f16, kind="ExternalInput", name=k)
                   for k, s in wspec16.items()}
            for k in ("eW2", "eW3", "eW4"):
                d_w[k] = dram.tile([P, 8192], f8, kind="ExternalInput", name=k)
            d_w["wcombo"] = dram.tile([P, 4096], f8, kind="ExternalInput", name="wcombo")
            d_w["n1W2"] = dram.tile([P, 2048], f8, kind="ExternalInput", name="n1W2")
            bspec = dict(eb2r64=[P, 8], eb3r=[P, 8], eb4r64=[P, 8], n2b2r=[1, 1])
            d_b = {k: dram.tile(s, f32, kind="ExternalInput", name=k)
                   for k, s in bspec.items()}
            d_z = dram.tile([1, NPAD], f32, kind="ExternalOutput", name="zout")

            names = dict(ein=d_ein.name, relw=d_relw.name, invc=d_invc.name,
                         xT2=d_xT2.name, iota=d_iota.name, n1b2bc=d_n1b2bc.name,
                         zout=d_z.name)
            names.update({k: v.name for k, v in d_w.items()})
            names.update({k: v.name for k, v in d_b.items()})

            # ---- resident loads --------------------------------------------
            # SP carries only the streamed ein blocks; weights alternate
            # between the ACT and Pool queues in the order compute needs
            # them, so each layer's pairs transfer on two queues in parallel.
            qs = [nc.scalar, nc.gpsimd]

            W1 = wres.tile([P, E_HID], f16, name="w_eW1p")
            nc.scalar.dma_start(out=W1[:], in_=d_w["eW1p"][:])

            def load_wdr(name, npair, width):
                ts = []
                for q in range(npair):
                    t = wres.tile([P, 2, width], f8, name=f"w_{name}_{q}")
                    qs[q % 2].dma_start(
                        out=t[:, :, :],
                        in_=d_w[name][:, q * 2 * width:(q + 1) * 2 * width])
                    ts.append(t)
                return ts

            W2 = load_wdr("eW2", 4, E_HID)
            W3 = load_wdr("eW3", 4, E_HID)
            W4 = load_wdr("eW4", 4, E_HID)
            Wc = load_wdr("wcombo", 4, N_HID)
            W7 = load_wdr("n1W2", 2, N_HID)
            W6a = wres.tile([P, N_HID], f16, name="w_n1W1a")
            nc.scalar.dma_start(out=W6a[:], in_=d_w["n1W1a"][:])
            B = {}
            for k, s in bspec.items():
                t = wres.tile(s, f32, name=f"b_{k}")
                nc.scalar.dma_start(out=t[:], in_=d_b[k][:])
                B[k] = t
            iota = wres.tile([P, WN], f32, name="iota_sb")
            nc.scalar.dma_start(out=iota[:], in_=d_iota[:])
            invc = wres.tile([P, T], f32, name="invc_sb")
            nc.scalar.dma_start(out=invc[:], in_=d_invc[:])
            n1b2bc = wres.tile([P, N_HID], f32, name="n1b2bc_sb")
            nc.scalar.dma_start(out=n1b2bc[:], in_=d_n1b2bc[:])
            # Deferred loads (first needed by window 0, ~45us in): emitted on
            # the Pool queue behind a gate op so their ~2MB of HBM traffic
            # does not compete with the critical W2..W7 transfers. The gate
            # input is filled in during block 0 emission.
            W8x = wres.tile([P, N_HID], f16, name="w_n2W1x")
            W8a = [wres.tile([P, N_HID], f16, name=f"w_n2W1agg_{k}")
                   for k in range(4)]
            W9 = [wres.tile([P, 1], f16, name=f"w_n2W2_{k}") for k in range(4)]
            relw = wres.tile([P, NWIN * T], f32, name="relw_sb")
            xT2 = wres.tile([P, NPAD], f16, name="xT2_sb")
            gdummy = wres.tile([1, 1], f16, name="gdummy")

            def emit_deferred_loads(gate_src):
                nc.gpsimd.tensor_scalar(out=gdummy[:], in0=gate_src,
                                        scalar1=0.0, scalar2=None, op0=ADD)
                nc.gpsimd.dma_start(out=W8x[:], in_=d_w["n2W1x"][:])
                for k in range(4):
                    nc.gpsimd.dma_start(out=W8a[k][:],
                                        in_=d_w["n2W1agg"][k * P:(k + 1) * P, :])
                for k in range(4):
                    nc.gpsimd.dma_start(out=W9[k][:],
                                        in_=d_w["n2W2"][k * P:(k + 1) * P, :])
                nc.gpsimd.dma_start(out=relw[:], in_=d_relw[:])
                nc.gpsimd.dma_start(out=xT2[:], in_=d_xT2[:])

            # h2 pair tiles by pair index (edge tiles 2k, 2k+1)
            h2_pairs = {}

            def dr_layer(pin, Wp, quant, tag):
                """fp8 DoubleRow layer; quant(out_plane, psum, m) applies the
                bias+relu+store for m-tile m."""
                outs = []
                pt = None
                for m in range(8):
                    ps = bigps.tile([P, 512], mybir.dt.float32, name="ps_big")
                    for q in range(len(Wp)):
                        nc.tensor.matmul(
                            out=ps[:], lhsT=Wp[q][:, :, m * P:(m + 1) * P],
                            rhs=pin[q][:, :, :], start=(q == 0),
                            stop=(q == len(Wp) - 1), perf_mode=DR)
                    if m % 2 == 0:
                        pt = actp.tile([P, 2, 512], f8, name=tag)
                        outs.append(pt)
                    quant(pt[:, m % 2, :], ps, m)
                return outs

            def q_dve64(btile):
                def q(o, ps, m):
                    nc.vector.tensor_scalar(
                        out=o, in0=ps[:], scalar1=btile[:, m:m + 1], scalar2=0.0,
                        op0=ADD, op1=MAX)
                return q

            def q_act(btile, scale):
                def q(o, ps, m):
                    nc.scalar.activation(o, ps[:], RELU,
                                         bias=btile[:, m:m + 1], scale=scale)
                return q

            def emit_block(b):
                e0 = b * 1024
                ein_t = einp.tile([P, 1024], f16, name="ein_t")
                nc.sync.dma_start(out=ein_t[:], in_=d_ein[:, e0:e0 + 1024])
                for h in range(2):
                    einh = ein_t[:, h * 512:(h + 1) * 512]
                    # L1 -> a1 (x1) on ACT; bias rides the ones-row of ein
                    a1p = []
                    pt = None
                    for m in range(8):
                        ps = bigps.tile([P, 512], mybir.dt.float32, name="ps_big")
                        nc.tensor.matmul(out=ps[:], lhsT=W1[:, m * P:(m + 1) * P],
                                         rhs=einh, start=True, stop=True)
                        if m % 2 == 0:
                            pt = actp.tile([P, 2, 512], f8, name="pairA")
                            a1p.append(pt)
                        nc.scalar.activation(pt[:, m % 2, :], ps[:], RELU)
                    # L2 -> a2 (x64) on DVE
                    a2p = dr_layer(a1p, W2, q_dve64(B["eb2r64"]), "pairB")
                    # L3 -> a3 (x1) on ACT
                    a3p = dr_layer(a2p, W3, q_act(B["eb3r"], 1.0 / 4096), "pairA")
                    # L4 -> a4 (x64) on DVE
                    a4p = dr_layer(a3p, W4, q_dve64(B["eb4r64"]), "pairB")
                    # combo (eW5 @ n1W1 fused) + fp16 x-part -> h1 (x64) on ACT
                    h1p = []
                    pt = None
                    for m in range(4):
                        ps = bigps.tile([P, 512], mybir.dt.float32, name="ps_big")
                        nc.tensor.matmul(out=ps[:], lhsT=W6a[:, m * P:(m + 1) * P],
                                         rhs=einh, start=True, stop=False)
                        for q in range(4):
                            nc.tensor.matmul(
                                out=ps[:], lhsT=Wc[q][:, :, m * P:(m + 1) * P],
                                rhs=a4p[q][:, :, :], start=False, stop=(q == 3),
                                perf_mode=DR)
                        if m % 2 == 0:
                            pt = actp.tile([P, 2, 512], f8, name="pairC")
                            h1p.append(pt)
                        nc.scalar.activation(pt[:, m % 2, :], ps[:], RELU,
                                             scale=1.0 / 64)
                    # n1 L2: edge-major h2 per 128-edge subtile -> fp8 pairs
                    for s in range(4):
                        t_glob = b * 8 + h * 4 + s
                        ps = bigps.tile([P, 512], mybir.dt.float32, name="ps_big")
                        for q in range(2):
                            nc.tensor.matmul(
                                out=ps[:], lhsT=h1p[q][:, :, s * P:(s + 1) * P],
                                rhs=W7[q][:, :, :], start=(q == 0), stop=(q == 1),
                                perf_mode=DR)
                        tmp = smal.tile([P, 512], f16, name="tmp16")
                        nc.vector.tensor_tensor(out=tmp[:], in0=ps[:],
                                                in1=n1b2bc[:], op=ADD)
                        if b == 0 and h == 1 and s == 0:
                            gate_ref[0] = tmp[0:1, 0:1]
                        k, j = t_glob // 2, t_glob % 2
                        if j == 0:
                            h2_pairs[k] = h2p.tile([P, 2, N_HID], f8, name="h2t")
                        nc.vector.tensor_scalar(
                            out=h2_pairs[k][:, j, :], in0=tmp[:],
                            scalar1=invc[:, t_glob:t_glob + 1], scalar2=0.0,
                            op0=MULT, op1=MAX)

            win_agg = {}

            def emit_window_seg(w):
                klo, khi = win_pairs[w]
                seg = segps.tile([P, 4, WN], mybir.dt.float32, name="segps_t")
                pl = list(range(klo, khi + 1))
                for ki_, k in enumerate(pl):
                    memb = smal.tile([P, 2, WN], f8, name="memb")
                    for j in range(2):
                        t = 2 * k + j
                        nc.vector.tensor_scalar(
                            out=memb[:, j, :], in0=iota[:],
                            scalar1=relw[:, w * T + t:w * T + t + 1], scalar2=None,
                            op0=ISEQ)
                    for fc in range(4):
                        nc.tensor.matmul(
                            out=seg[:, fc, :],
                            lhsT=h2_pairs[k][:, :, fc * P:(fc + 1) * P],
                            rhs=memb[:, :, :], start=(ki_ == 0),
                            stop=(ki_ == len(pl) - 1), perf_mode=DR)
                agg = []
                for fc in range(4):
                    at = aggp.tile([P, WN], f16, name="aggt")
                    nc.scalar.activation(at[:], seg[:, fc, :], COPY)
                    agg.append(at)
                win_agg[w] = agg

            def emit_window_mlp(w):
                agg = win_agg.pop(w)
                # ---- node MLP2 on this 256-node window ----
                n0 = w * WN
                xin = [xT2[:, n0:n0 + WN]] + [a[:] for a in agg]
                Win = [W8x] + W8a
                z1 = []
                for m in range(4):
                    ps = bigps.tile([P, 512], mybir.dt.float32, name="ps_big")
                    pss = ps[:, :WN]
                    for ki in range(5):
                        nc.tensor.matmul(out=pss, lhsT=Win[ki][:, m * P:(m + 1) * P],
                                         rhs=xin[ki], start=(ki == 0), stop=(ki == 4))
                    zt = smal.tile([P, WN], f16, name="z1t")
                    nc.scalar.activation(zt[:], pss, RELU)
                    z1.append(zt)
                ps = bigps.tile([P, 512], mybir.dt.float32, name="ps_big")
                pss = ps[:1, :WN]
                for ki in range(4):
                    nc.tensor.matmul(out=pss, lhsT=W9[ki][:], rhs=z1[ki][:],
                                     start=(ki == 0), stop=(ki == 3))
                zo = smal.tile([1, WN], f32, name="zot")
                nc.vector.tensor_scalar(out=zo[:], in0=pss,
                                        scalar1=B["n2b2r"][0:1, 0:1], scalar2=None,
                                        op0=ADD)
                nc.sync.dma_start(out=d_z[:, n0:n0 + WN], in_=zo[:])

            # window w ready once tile win_tiles[w][1] is produced; its seg
            # matmuls queue right after that block, the mlp2 part one block
            # later so PE has block b+1 work while ACT copies agg out.
            ready = {}
            for w in range(NWIN):
                b_ready = min(NB - 1, win_tiles[w][1] // 8)
                ready.setdefault(b_ready, []).append(w)
            gate_ref = [None]
            pending_mlp = []
            for b in range(NB):
                emit_block(b)
                if b == 0:
                    emit_deferred_loads(gate_ref[0])
                for w in pending_mlp:
                    emit_window_mlp(w)
                pending_mlp = []
                for w in ready.get(b, []):
                    emit_window_seg(w)
                    pending_mlp.append(w)
            for w in pending_mlp:
                emit_window_mlp(w)

    nc.compile()
    return nc, names


def kernel(x, edge_attr, u, edge_index, batch, Wsel, bsel,
           eW1, eb1, eW2, eb2, eW3, eb3, eW4, eb4, eW5, eb5,
           n1W1, n1b1, n1W2, n1b2, n2W1, n2b1, n2W2, n2b2):
    f32 = np.float32
    f16 = np.float16
    x = np.asarray(x, f32)
    edge_attr = np.asarray(edge_attr, f32)
    u = np.asarray(u, f32)
    edge_index = np.asarray(edge_index)
    batch = np.asarray(batch)
    ws = {k: np.asarray(v, f32) for k, v in dict(
        Wsel=Wsel, bsel=bsel, eW1=eW1, eb1=eb1, eW2=eW2, eb2=eb2, eW3=eW3,
        eb3=eb3, eW4=eW4, eb4=eb4, eW5=eW5, eb5=eb5, n1W1=n1W1, n1b1=n1b1,
        n1W2=n1W2, n1b2=n1b2, n2W1=n2W1, n2b1=n2b1, n2W2=n2W2, n2b2=n2b2).items()}

    # ---------------- host math (index plumbing + tiny matmuls) ------------
    u2 = (u @ ws["Wsel"] + ws["bsel"]).astype(f32)          # [64, 256]
    row = np.asarray(edge_index[0], np.int64)
    col = np.asarray(edge_index[1], np.int64)
    order = np.argsort(row, kind="stable")
    row_s, col_s = row[order], col[order]
    g_s = np.asarray(batch[row_s], np.int64)
    ea_s = edge_attr[order, 0]
    cnt = np.bincount(row, minlength=N_NODES).astype(f32)
    invc_node = (1.0 / np.maximum(cnt, 1.0)).astype(f32)

    bounds = np.searchsorted(row_s, np.arange(0, N_NODES + 1, NPN))
    e_cnt = np.diff(bounds)
    EPAD = int(-(-int(e_cnt.max()) // 1024) * 1024)
    T = EPAD // P

    # per-window tile ranges (shared across cores) for the static program
    tlo = np.full(NWIN, T - 1, np.int64)
    thi = np.zeros(NWIN, np.int64)
    core_dat = []
    for c in range(NC):
        lo, hi = bounds[c], bounds[c + 1]
        n = hi - lo
        rel = np.full(EPAD, 1e6, f32)
        rel[:n] = (row_s[lo:hi] - NPN * c).astype(f32)
        w_of_edge = np.floor_divide(rel[:n].astype(np.int64), WN)
        for w in range(NWIN):
            idx = np.nonzero(w_of_edge == w)[0]
            if idx.size:
                tlo[w] = min(tlo[w], idx[0] // P)
                thi[w] = max(thi[w], idx[-1] // P)
        core_dat.append((lo, hi, n, rel))
    win_tiles = [(int(tlo[w]), int(max(tlo[w], thi[w]))) for w in range(NWIN)]
    h2_bufs = max(th // 2 - tl // 2 + 1 for tl, th in win_tiles) + 10

    # ---------------- shared weight packing ---------------------------------
    G64 = u2 @ ws["eW1"][19:275]                      # [64, 1024]
    eW1p = np.zeros((P, E_HID), f32)
    eW1p[0:9] = ws["eW1"][9:18]       # x[col] (dest)
    eW1p[9:18] = ws["eW1"][0:9]       # x[row] (src)
    eW1p[18] = ws["eW1"][18]          # edge_attr
    eW1p[19:83] = G64                 # one-hot graph -> u2 @ eW1
    eW1p[83] = ws["eb1"]              # ones-row bias

    Wc_mat = ws["eW5"] @ ws["n1W1"][9:521]            # [1024, 512]
    b_combo = ws["eb5"] @ ws["n1W1"][9:521] + ws["n1b1"]
    n1W1a = np.zeros((P, N_HID), f32)
    n1W1a[0:9] = ws["n1W1"][0:9] * 4096.0
    n1W1a[83] = b_combo * 4096.0

    H64 = u2 @ ws["n2W1"][521:777]                    # [64, 512]
    n2W1x = np.zeros((P, N_HID), f32)
    n2W1x[0:9] = ws["n2W1"][0:9]
    n2W1x[9:73] = H64
    n2W1x[73] = ws["n2b1"]

    def br(b, nm):   # bias [nm*128] -> [128, nm]
        return np.ascontiguousarray(b.reshape(nm, P).T).astype(f32)

    import ml_dtypes
    fp8 = ml_dtypes.float8_e4m3

    def packdr(W):   # [K, M] -> [128, (K//128)*M] fp8, x64, (q,j,m) free order
        K, M = W.shape
        Wp = (W * 64.0).reshape(K // 256, 2, P, M)
        return np.ascontiguousarray(
            np.transpose(Wp, (2, 0, 1, 3)).reshape(P, (K // P) * M)).astype(fp8)

    shared = dict(
        eW1p=eW1p.astype(f16), eW2=packdr(ws["eW2"]),
        eW3=packdr(ws["eW3"]), eW4=packdr(ws["eW4"]),
        wcombo=packdr(Wc_mat), n1W2=packdr(ws["n1W2"]),
        n1W1a=n1W1a.astype(f16), n2W1x=n2W1x.astype(f16),
        n2W1agg=(ws["n2W1"][9:521] / 64.0).astype(f16),
        n2W2=ws["n2W2"].astype(f16),
        eb2r64=br(ws["eb2"] * 64.0, 8), eb3r=br(ws["eb3"], 8),
        eb4r64=br(ws["eb4"] * 64.0, 8),
        n2b2r=ws["n2b2"].reshape(1, 1).astype(f32),
        iota=np.tile(np.arange(WN, dtype=f32), (P, 1)),
        n1b2bc=np.tile(ws["n1b2"].astype(f32) * 4096.0, (P, 1)),
    )

    # ---------------- per-core input packing --------------------------------
    in_maps = []
    for c in range(NC):
        lo, hi, n, rel = core_dat[c]
        ein = np.zeros((P, EPAD), f32)
        ein[0:9, :n] = x[col_s[lo:hi]].T
        ein[9:18, :n] = x[row_s[lo:hi]].T
        ein[18, :n] = ea_s[lo:hi]
        ein[19 + g_s[lo:hi], np.arange(n)] = 1.0
        ein[83, :n] = 1.0
        relw = np.empty((P, NWIN * T), f32)
        for w in range(NWIN):
            relw[:, w * T:(w + 1) * T] = _pack_cols(rel - float(WN) * w, T)
        invc_e = np.ones(EPAD, f32)
        invc_e[:n] = invc_node[row_s[lo:hi]]
        invc_e *= 1.0 / 64.0
        xT2 = np.zeros((P, NPAD), f32)
        xT2[0:9, :NPN] = x[NPN * c:NPN * (c + 1)].T
        bc = np.asarray(batch[NPN * c:NPN * (c + 1)], np.int64)
        xT2[9 + bc, np.arange(NPN)] = 1.0
        xT2[73, :NPN] = 1.0
        im = dict(shared)
        im.update(ein=ein.astype(f16), relw=relw,
                  invc=_pack_cols(invc_e, T), xT2=xT2.astype(f16))
        in_maps.append(im)

    # ---------------- build + run ------------------------------------------
    key = (EPAD, tuple(win_tiles))
    if key not in _CACHE:
        _CACHE[key] = _build_module(EPAD, win_tiles, h2_bufs)
    nc, names = _CACHE[key]

    from concourse import bass_utils
    trace = bool(int(os.environ.get("KERNEL_TRACE", "0")))
    if trace:
        try:
            import types
            import antenv
            if not hasattr(antenv, "axon_hooks"):
                mod = types.ModuleType("antenv.axon_hooks")
                mod._hook = None
                mod.set_axon_ntff_profile_hook = lambda h: setattr(mod, "_hook", h)
                mod.get_axon_ntff_profile_hook = lambda: mod._hook
                sys.modules["antenv.axon_hooks"] = mod
                antenv.axon_hooks = mod
                from trn_agent_boot.trn_boot import _ntff_profile_via_ctypes
                mod._hook = _ntff_profile_via_ctypes("/opt/axon/libaxon_pjrt.so")
        except Exception as e:  # profiling is best-effort
            print("ntff hook shim failed:", e)
            trace = False
    real_maps = [{names[k]: v for k, v in im.items()} for im in in_maps]
    res = bass_utils.run_bass_kernel_spmd(
        nc, real_maps, core_ids=list(range(NC)), trace=trace)
    if trace and res.exec_time_ns is not None:
        print(f"HW exec time: {res.exec_time_ns} ns")
        if res.instructions_and_trace:
            print("trace:", res.instructions_and_trace[1])

    out = np.empty(N_NODES, f32)
    for c in range(NC):
        out[NPN * c:NPN * (c + 1)] = res.results[c][names["zout"]][0, :NPN]
    return out


# revision 21
# speedup vs baseline: 1.2159x; 1.0008x over previous
"""OGRENet GNN message-passing kernel for 8 Trainium2 NeuronCores.

Strategy
--------
Host (numpy, cheap index plumbing only):
  * u2 = u @ Wsel + bsel (64x256), G64 = u2 @ eW1[19:275] (64x1024),
    H64 = u2 @ n2W1[521:777] (64x512): the per-edge/node u2 gather becomes a
    64-wide one-hot riding inside the K=128 first-layer matmul.
  * W_combo = eW5 @ n1W1[9:521]: e5 is never materialized (it is only ever
    consumed linearly by node MLP1), fusing two layers into one.
  * sort edges by destination node (`row`), split into 8 contiguous chunks at
    node boundaries -> each core owns a contiguous node range and ALL edges
    that scatter into it => no cross-core reduction.
  * pack per-core feature-major edge inputs ein = [x[col]; x[row]; e_attr;
    onehot(g); 1] (fp16, 128 rows total), per-edge 1/count scales, and
    window-relative row ids.

Device (per core, identical program, different data):
  * edge MLP layers as fp8 DoubleRow matmuls (157 TF/s), fp32 PSUM. Biases and
    relu are applied in a scale-staged way so the quantize ops spread across
    ACT/DVE/GpSimd and the tensor engine never waits:
      a1 = relu(ps)            x1   ACT   (bias via ones-row in ein)
      a2 = max(ps + 64 b2, 0)  x64  DVE   (2-op tensor_scalar)
      a3 = relu(ps/4096 + b3)  x1   ACT
      a4 = max(ps + 64 b4, 0)  x64  DVE
      h1 = relu(ps/64)         x64  ACT   (bias via ones-row in x-part)
      h2 = max((ps+4096 b) * invc/64, 0) fp8 x64  GpSimd add + DVE mul/relu
  * segment-sum as matmul with fp8 DoubleRow: membership pairs M[e,n]
    (is_equal on DVE) contract TWO 128-edge tiles per pass into a
    [feat, 256-node] PSUM window.
  * node MLP2 consumes agg feature-major; u2[batch] + bias enter through
    one-hot/ones rows of the x input tile; z -> DRAM.
"""

import os
import sys

import numpy as np

sys.path.insert(0, "/opt/trn_rl_repo")

N_NODES = 20000
N_GRAPHS = 64
E_HID = 1024
N_HID = 512
NC = 8
NPN = N_NODES // NC          # nodes per core (2500)
NPAD = 2560                  # padded nodes per core
WN = 256                     # nodes per segment window
NWIN = NPAD // WN            # 10
P = 128

_CACHE = {}


def _pack_cols(v, T):
    """[T*128] -> [128, T] with col t = v[t*128:(t+1)*128]."""
    return np.ascontiguousarray(v.reshape(T, P).T)


def _build_module(EPAD, win_tiles, h2_bufs):
    """Build the per-core Bass program. win_tiles[w] = (tlo, thi) inclusive."""
    from concourse import bacc, mybir, tile

    T = EPAD // P           # 128-edge tiles
    NB = EPAD // 1024       # 1024-edge blocks
    win_pairs = [(tl // 2, th // 2) for tl, th in win_tiles]
    f16 = mybir.dt.float16
    f32 = mybir.dt.float32
    f8 = mybir.dt.float8e4
    RELU = mybir.ActivationFunctionType.Relu
    COPY = mybir.ActivationFunctionType.Copy
    ADD = mybir.AluOpType.add
    MULT = mybir.AluOpType.mult
    MAX = mybir.AluOpType.max
    ISEQ = mybir.AluOpType.is_equal
    DR = mybir.MatmulPerfMode.DoubleRow

    nc = bacc.Bacc(None, target_bir_lowering=False, debug=False)

    with tile.TileContext(nc) as tc:
        with (
            tc.tile_pool(name="dram", bufs=1, space="DRAM") as dram,
            tc.tile_pool(name="wres", bufs=1) as wres,
            tc.tile_pool(name="einp", bufs=4) as einp,
            tc.tile_pool(name="actp", bufs=14) as actp,
            tc.tile_pool(name="h2p", bufs=h2_bufs) as h2p,
            tc.tile_pool(name="smal", bufs=8) as smal,
            tc.tile_pool(name="aggp", bufs=12) as aggp,
            tc.tile_pool(name="bigps", bufs=6, space="PSUM") as bigps,
            tc.tile_pool(name="segps", bufs=1, space="PSUM") as segps,
        ):
            # ---- DRAM I/O -------------------------------------------------
            d_ein = dram.tile([P, EPAD], f16, kind="ExternalInput", name="ein")
            d_relw = dram.tile([P, NWIN * T], f32, kind="ExternalInput", name="relw")
            d_invc = dram.tile([P, T], f32, kind="ExternalInput", name="invc")
            d_xT2 = dram.tile([P, NPAD], f16, kind="ExternalInput", name="xT2")
            d_iota = dram.tile([P, WN], f32, kind="ExternalInput", name="iota")
            d_n1b2bc = dram.tile([P, N_HID], f32, kind="ExternalInput", name="n1b2bc")

            wspec16 = dict(eW1p=[P, E_HID], n1W1a=[P, N_HID], n2W1x=[P, N_HID],
                           n2W2=[N_HID, 1])
            d_w = {k: dram.tile(s, f16, kind="ExternalInput", name=k)
                   for k, s in wspec16.items()}
            for k in ("eW2", "eW3", "eW4"):
                d_w[k] = dram.tile([P, 8192], f8, kind="ExternalInput", name=k)
            d_w["wcombo"] = dram.tile([P, 4096], f8, kind="ExternalInput", name="wcombo")
            d_w["n1W2"] = dram.tile([P, 2048], f8, kind="ExternalInput", name="n1W2")
            d_w["n2W1agg"] = dram.tile([P, 2048], f8, kind="ExternalInput",
                                       name="n2W1agg")
            bspec = dict(eb2r64=[P, 8], eb3r=[P, 8], eb4r64=[P, 8], n2b2r=[1, 1])
            d_b = {k: dram.tile(s, f32, kind="ExternalInput", name=k)
                   for k, s in bspec.items()}
            d_z = dram.tile([1, NPAD], f32, kind="ExternalOutput", name="zout")

            names = dict(ein=d_ein.name, relw=d_relw.name, invc=d_invc.name,
                         xT2=d_xT2.name, iota=d_iota.name, n1b2bc=d_n1b2bc.name,
                         zout=d_z.name)
            names.update({k: v.name for k, v in d_w.items()})
            names.update({k: v.name for k, v in d_b.items()})

            # ---- streamed ein blocks (SP queue, blocks 0/1 hoisted first) --
            ein_tiles = {}

            def fetch_ein(b):
                t = einp.tile([P, 1024], f16, name="ein_t")
                nc.sync.dma_start(out=t[:], in_=d_ein[:, b * 1024:(b + 1) * 1024])
                ein_tiles[b] = t

            fetch_ein(0)
            fetch_ein(1)

            # ---- resident loads --------------------------------------------
            # Critical weights round-robin across all three DMA-capable
            # queues in the order compute needs them so the aggregate HBM
            # stream delivers each layer just in time.
            qrr = [nc.gpsimd, nc.sync, nc.scalar]
            qn = [0]

            def nextq():
                qn[0] += 1
                return qrr[qn[0] % 3]

            W1 = wres.tile([P, E_HID], f16, name="w_eW1p")
            nc.scalar.dma_start(out=W1[:], in_=d_w["eW1p"][:])

            def load_wdr(name, npair, width):
                ts = []
                for q in range(npair):
                    t = wres.tile([P, 2, width], f8, name=f"w_{name}_{q}")
                    nextq().dma_start(
                        out=t[:, :, :],
                        in_=d_w[name][:, q * 2 * width:(q + 1) * 2 * width])
                    ts.append(t)
                return ts

            W2 = load_wdr("eW2", 4, E_HID)
            W3 = load_wdr("eW3", 4, E_HID)
            W4 = load_wdr("eW4", 4, E_HID)
            Wc = load_wdr("wcombo", 4, N_HID)
            W7 = load_wdr("n1W2", 2, N_HID)
            W6a = wres.tile([P, N_HID], f16, name="w_n1W1a")
            nc.scalar.dma_start(out=W6a[:], in_=d_w["n1W1a"][:])
            B = {}
            for k, s in bspec.items():
                t = wres.tile(s, f32, name=f"b_{k}")
                nc.scalar.dma_start(out=t[:], in_=d_b[k][:])
                B[k] = t
            iota = wres.tile([P, WN], f32, name="iota_sb")
            nc.scalar.dma_start(out=iota[:], in_=d_iota[:])
            invc = wres.tile([P, T], f32, name="invc_sb")
            nc.scalar.dma_start(out=invc[:], in_=d_invc[:])
            n1b2bc = wres.tile([P, N_HID], f32, name="n1b2bc_sb")
            nc.scalar.dma_start(out=n1b2bc[:], in_=d_n1b2bc[:])
            # Deferred loads (first needed by window 0, ~45us in): emitted on
            # the Pool queue behind a gate op so their ~2MB of HBM traffic
            # does not compete with the critical W2..W7 transfers. The gate
            # input is filled in during block 0 emission.
            W8x = wres.tile([P, N_HID], f16, name="w_n2W1x")
            W8a = [wres.tile([P, 2, N_HID], f8, name=f"w_n2W1agg_{q}")
                   for q in range(2)]
            W9 = [wres.tile([P, 1], f16, name=f"w_n2W2_{k}") for k in range(4)]
            relw = wres.tile([P, NWIN * T], f32, name="relw_sb")
            xT2 = wres.tile([P, NPAD], f16, name="xT2_sb")
            gdummy = wres.tile([1, 1], f16, name="gdummy")

            def emit_deferred_loads(gate_src):
                nc.gpsimd.tensor_scalar(out=gdummy[:], in0=gate_src,
                                        scalar1=0.0, scalar2=None, op0=ADD)
                nc.gpsimd.dma_start(out=W8x[:], in_=d_w["n2W1x"][:])
                for q in range(2):
                    nc.gpsimd.dma_start(
                        out=W8a[q][:, :, :],
                        in_=d_w["n2W1agg"][:, q * 1024:(q + 1) * 1024])
                for k in range(4):
                    nc.gpsimd.dma_start(out=W9[k][:],
                                        in_=d_w["n2W2"][k * P:(k + 1) * P, :])
                nc.gpsimd.dma_start(out=relw[:], in_=d_relw[:])
                nc.gpsimd.dma_start(out=xT2[:], in_=d_xT2[:])

            # h2 pair tiles by pair index (edge tiles 2k, 2k+1)
            h2_pairs = {}

            def dr_layer(pin, Wp, quant, tag):
                """fp8 DoubleRow layer; quant(out_plane, psum, m) applies the
                bias+relu+store for m-tile m."""
                outs = []
                pt = None
                for m in range(8):
                    ps = bigps.tile([P, 512], mybir.dt.float32, name="ps_big")
                    for q in range(len(Wp)):
                        nc.tensor.matmul(
                            out=ps[:], lhsT=Wp[q][:, :, m * P:(m + 1) * P],
                            rhs=pin[q][:, :, :], start=(q == 0),
                            stop=(q == len(Wp) - 1), perf_mode=DR)
                    if m % 2 == 0:
                        pt = actp.tile([P, 2, 512], f8, name=tag)
                        outs.append(pt)
                    quant(pt[:, m % 2, :], ps, m)
                return outs

            def q_dve64(btile):
                def q(o, ps, m):
                    nc.vector.tensor_scalar(
                        out=o, in0=ps[:], scalar1=btile[:, m:m + 1], scalar2=0.0,
                        op0=ADD, op1=MAX)
                return q

            def q_act(btile, scale):
                def q(o, ps, m):
                    nc.scalar.activation(o, ps[:], RELU,
                                         bias=btile[:, m:m + 1], scale=scale)
                return q

            def emit_block(b):
                if b + 2 < NB:
                    fetch_ein(b + 2)
                ein_t = ein_tiles.pop(b)
                for h in range(2):
                    einh = ein_t[:, h * 512:(h + 1) * 512]
                    # L1 -> a1 (x1) on ACT; bias rides the ones-row of ein
                    a1p = []
                    pt = None
                    for m in range(8):
                        ps = bigps.tile([P, 512], mybir.dt.float32, name="ps_big")
                        nc.tensor.matmul(out=ps[:], lhsT=W1[:, m * P:(m + 1) * P],
                                         rhs=einh, start=True, stop=True)
                        if m % 2 == 0:
                            pt = actp.tile([P, 2, 512], f8, name="pairA")
                            a1p.append(pt)
                        nc.scalar.activation(pt[:, m % 2, :], ps[:], RELU)
                    # L2 -> a2 (x64) on DVE
                    a2p = dr_layer(a1p, W2, q_dve64(B["eb2r64"]), "pairB")
                    # L3 -> a3 (x1) on ACT
                    a3p = dr_layer(a2p, W3, q_act(B["eb3r"], 1.0 / 4096), "pairA")
                    # L4 -> a4 (x64) on DVE
                    a4p = dr_layer(a3p, W4, q_dve64(B["eb4r64"]), "pairB")
                    # combo (eW5 @ n1W1 fused) + fp16 x-part -> h1 (x64) on ACT
                    h1p = []
                    pt = None
                    for m in range(4):
                        ps = bigps.tile([P, 512], mybir.dt.float32, name="ps_big")
                        nc.tensor.matmul(out=ps[:], lhsT=W6a[:, m * P:(m + 1) * P],
                                         rhs=einh, start=True, stop=False)
                        for q in range(4):
                            nc.tensor.matmul(
                                out=ps[:], lhsT=Wc[q][:, :, m * P:(m + 1) * P],
                                rhs=a4p[q][:, :, :], start=False, stop=(q == 3),
                                perf_mode=DR)
                        if m % 2 == 0:
                            pt = actp.tile([P, 2, 512], f8, name="pairC")
                            h1p.append(pt)
                        nc.scalar.activation(pt[:, m % 2, :], ps[:], RELU,
                                             scale=1.0 / 64)
                    # n1 L2: edge-major h2 per 128-edge subtile -> fp8 pairs
                    for s in range(4):
                        t_glob = b * 8 + h * 4 + s
                        ps = bigps.tile([P, 512], mybir.dt.float32, name="ps_big")
                        for q in range(2):
                            nc.tensor.matmul(
                                out=ps[:], lhsT=h1p[q][:, :, s * P:(s + 1) * P],
                                rhs=W7[q][:, :, :], start=(q == 0), stop=(q == 1),
                                perf_mode=DR)
                        tmp = smal.tile([P, 512], f16, name="tmp16")
                        nc.vector.tensor_tensor(out=tmp[:], in0=ps[:],
                                                in1=n1b2bc[:], op=ADD)
                        if b == 0 and h == 1 and s == 0:
                            gate_ref[0] = tmp[0:1, 0:1]
                        k, j = t_glob // 2, t_glob % 2
                        if j == 0:
                            h2_pairs[k] = h2p.tile([P, 2, N_HID], f8, name="h2t")
                        nc.vector.tensor_scalar(
                            out=h2_pairs[k][:, j, :], in0=tmp[:],
                            scalar1=invc[:, t_glob:t_glob + 1], scalar2=0.0,
                            op0=MULT, op1=MAX)

            win_agg = {}

            def emit_window_seg(w):
                klo, khi = win_pairs[w]
                seg = segps.tile([P, 4, WN], mybir.dt.float32, name="segps_t")
                pl = list(range(klo, khi + 1))
                for ki_, k in enumerate(pl):
                    memb = smal.tile([P, 2, WN], f8, name="memb")
                    for j in range(2):
                        t = 2 * k + j
                        nc.vector.tensor_scalar(
                            out=memb[:, j, :], in0=iota[:],
                            scalar1=relw[:, w * T + t:w * T + t + 1], scalar2=None,
                            op0=ISEQ)
                    for fc in range(4):
                        nc.tensor.matmul(
                            out=seg[:, fc, :],
                            lhsT=h2_pairs[k][:, :, fc * P:(fc + 1) * P],
                            rhs=memb[:, :, :], start=(ki_ == 0),
                            stop=(ki_ == len(pl) - 1), perf_mode=DR)
                agg = []
                for p in range(2):
                    at = aggp.tile([P, 2, WN], f8, name="aggt")
                    agg.append(at)
                    for j in range(2):
                        nc.scalar.activation(at[:, j, :], seg[:, 2 * p + j, :], COPY)
                win_agg[w] = agg

            def emit_window_mlp(w):
                agg = win_agg.pop(w)
                # ---- node MLP2 on this 256-node window ----
                n0 = w * WN
                z1 = []
                for m in range(4):
                    ps = bigps.tile([P, 512], mybir.dt.float32, name="ps_big")
                    pss = ps[:, :WN]
                    nc.tensor.matmul(out=pss, lhsT=W8x[:, m * P:(m + 1) * P],
                                     rhs=xT2[:, n0:n0 + WN], start=True, stop=False)
                    for q in range(2):
                        nc.tensor.matmul(out=pss,
                                         lhsT=W8a[q][:, :, m * P:(m + 1) * P],
                                         rhs=agg[q][:, :, :], start=False,
                                         stop=(q == 1), perf_mode=DR)
                    zt = smal.tile([P, WN], f16, name="z1t")
                    nc.scalar.activation(zt[:], pss, RELU, scale=1.0 / 4096)
                    z1.append(zt)
                ps = bigps.tile([P, 512], mybir.dt.float32, name="ps_big")
                pss = ps[:1, :WN]
                for ki in range(4):
                    nc.tensor.matmul(out=pss, lhsT=W9[ki][:], rhs=z1[ki][:],
                                     start=(ki == 0), stop=(ki == 3))
                zo = smal.tile([1, WN], f32, name="zot")
                nc.vector.tensor_scalar(out=zo[:], in0=pss,
                                        scalar1=B["n2b2r"][0:1, 0:1], scalar2=None,
                                        op0=ADD)
                nc.sync.dma_start(out=d_z[:, n0:n0 + WN], in_=zo[:])

            # window w's h2 pairs complete during block win_tiles[w][1]//8;
            # its seg matmuls queue one block later (so the DVE h2 quantizes
            # have drained), the mlp2 part one block after that (so PE has
            # block work while ACT copies agg out).
            seg_at = {}
            for w in range(NWIN):
                br = min(NB - 1, win_tiles[w][1] // 8)
                sb = br + 1 if br + 1 <= NB - 2 else br
                seg_at.setdefault(sb, []).append(w)
            gate_ref = [None]
            mlp_at = {}
            post_mlp = []
            for b in range(NB):
                emit_block(b)
                if b == 0:
                    emit_deferred_loads(gate_ref[0])
                for w in mlp_at.pop(b, []):
                    emit_window_mlp(w)
                for w in seg_at.get(b, []):
                    emit_window_seg(w)
                    if b + 1 < NB:
                        mlp_at.setdefault(b + 1, []).append(w)
                    else:
                        post_mlp.append(w)
            for w in post_mlp:
                emit_window_mlp(w)

    nc.compile()
    return nc, names


def kernel(x, edge_attr, u, edge_index, batch, Wsel, bsel,
           eW1, eb1, eW2, eb2, eW3, eb3, eW4, eb4, eW5, eb5,
           n1W1, n1b1, n1W2, n1b2, n2W1, n2b1, n2W2, n2b2):
    f32 = np.float32
    f16 = np.float16
    x = np.asarray(x, f32)
    edge_attr = np.asarray(edge_attr, f32)
    u = np.asarray(u, f32)
    edge_index = np.asarray(edge_index)
    batch = np.asarray(batch)
    ws = {k: np.asarray(v, f32) for k, v in dict(
        Wsel=Wsel, bsel=bsel, eW1=eW1, eb1=eb1, eW2=eW2, eb2=eb2, eW3=eW3,
        eb3=eb3, eW4=eW4, eb4=eb4, eW5=eW5, eb5=eb5, n1W1=n1W1, n1b1=n1b1,
        n1W2=n1W2, n1b2=n1b2, n2W1=n2W1, n2b1=n2b1, n2W2=n2W2, n2b2=n2b2).items()}

    # ---------------- host math (index plumbing + tiny matmuls) ------------
    u2 = (u @ ws["Wsel"] + ws["bsel"]).astype(f32)          # [64, 256]
    row = np.asarray(edge_index[0], np.int64)
    col = np.asarray(edge_index[1], np.int64)
    order = np.argsort(row, kind="stable")
    row_s, col_s = row[order], col[order]
    g_s = np.asarray(batch[row_s], np.int64)
    ea_s = edge_attr[order, 0]
    cnt = np.bincount(row, minlength=N_NODES).astype(f32)
    invc_node = (1.0 / np.maximum(cnt, 1.0)).astype(f32)

    bounds = np.searchsorted(row_s, np.arange(0, N_NODES + 1, NPN))
    e_cnt = np.diff(bounds)
    EPAD = int(-(-int(e_cnt.max()) // 1024) * 1024)
    T = EPAD // P

    # per-window tile ranges (shared across cores) for the static program
    tlo = np.full(NWIN, T - 1, np.int64)
    thi = np.zeros(NWIN, np.int64)
    core_dat = []
    for c in range(NC):
        lo, hi = bounds[c], bounds[c + 1]
        n = hi - lo
        rel = np.full(EPAD, 1e6, f32)
        rel[:n] = (row_s[lo:hi] - NPN * c).astype(f32)
        w_of_edge = np.floor_divide(rel[:n].astype(np.int64), WN)
        for w in range(NWIN):
            idx = np.nonzero(w_of_edge == w)[0]
            if idx.size:
                tlo[w] = min(tlo[w], idx[0] // P)
                thi[w] = max(thi[w], idx[-1] // P)
        core_dat.append((lo, hi, n, rel))
    win_tiles = [(int(tlo[w]), int(max(tlo[w], thi[w]))) for w in range(NWIN)]
    h2_bufs = max(th // 2 - tl // 2 + 1 for tl, th in win_tiles) + 18

    # ---------------- shared weight packing ---------------------------------
    G64 = u2 @ ws["eW1"][19:275]                      # [64, 1024]
    eW1p = np.zeros((P, E_HID), f32)
    eW1p[0:9] = ws["eW1"][9:18]       # x[col] (dest)
    eW1p[9:18] = ws["eW1"][0:9]       # x[row] (src)
    eW1p[18] = ws["eW1"][18]          # edge_attr
    eW1p[19:83] = G64                 # one-hot graph -> u2 @ eW1
    eW1p[83] = ws["eb1"]              # ones-row bias

    Wc_mat = ws["eW5"] @ ws["n1W1"][9:521]            # [1024, 512]
    b_combo = ws["eb5"] @ ws["n1W1"][9:521] + ws["n1b1"]
    n1W1a = np.zeros((P, N_HID), f32)
    n1W1a[0:9] = ws["n1W1"][0:9] * 4096.0
    n1W1a[83] = b_combo * 4096.0

    H64 = u2 @ ws["n2W1"][521:777]                    # [64, 512]
    n2W1x = np.zeros((P, N_HID), f32)
    n2W1x[0:9] = ws["n2W1"][0:9]
    n2W1x[9:73] = H64
    n2W1x[73] = ws["n2b1"]
    n2W1x *= 4096.0

    def br(b, nm):   # bias [nm*128] -> [128, nm]
        return np.ascontiguousarray(b.reshape(nm, P).T).astype(f32)

    import ml_dtypes
    fp8 = ml_dtypes.float8_e4m3

    def packdr(W):   # [K, M] -> [128, (K//128)*M] fp8, x64, (q,j,m) free order
        K, M = W.shape
        Wp = (W * 64.0).reshape(K // 256, 2, P, M)
        return np.ascontiguousarray(
            np.transpose(Wp, (2, 0, 1, 3)).reshape(P, (K // P) * M)).astype(fp8)

    shared = dict(
        eW1p=eW1p.astype(f16), eW2=packdr(ws["eW2"]),
        eW3=packdr(ws["eW3"]), eW4=packdr(ws["eW4"]),
        wcombo=packdr(Wc_mat), n1W2=packdr(ws["n1W2"]),
        n1W1a=n1W1a.astype(f16), n2W1x=n2W1x.astype(f16),
        n2W1agg=packdr(ws["n2W1"][9:521]),
        n2W2=ws["n2W2"].astype(f16),
        eb2r64=br(ws["eb2"] * 64.0, 8), eb3r=br(ws["eb3"], 8),
        eb4r64=br(ws["eb4"] * 64.0, 8),
        n2b2r=ws["n2b2"].reshape(1, 1).astype(f32),
        iota=np.tile(np.arange(WN, dtype=f32), (P, 1)),
        n1b2bc=np.tile(ws["n1b2"].astype(f32) * 4096.0, (P, 1)),
    )

    # ---------------- per-core input packing --------------------------------
    in_maps = []
    for c in range(NC):
        lo, hi, n, rel = core_dat[c]
        ein = np.zeros((P, EPAD), f32)
        ein[0:9, :n] = x[col_s[lo:hi]].T
        ein[9:18, :n] = x[row_s[lo:hi]].T
        ein[18, :n] = ea_s[lo:hi]
        ein[19 + g_s[lo:hi], np.arange(n)] = 1.0
        ein[83, :n] = 1.0
        relw = np.empty((P, NWIN * T), f32)
        for w in range(NWIN):
            relw[:, w * T:(w + 1) * T] = _pack_cols(rel - float(WN) * w, T)
        invc_e = np.ones(EPAD, f32)
        invc_e[:n] = invc_node[row_s[lo:hi]]
        invc_e *= 1.0 / 64.0
        xT2 = np.zeros((P, NPAD), f32)
        xT2[0:9, :NPN] = x[NPN * c:NPN * (c + 1)].T
        bc = np.asarray(batch[NPN * c:NPN * (c + 1)], np.int64)
        xT2[9 + bc, np.arange(NPN)] = 1.0
        xT2[73, :NPN] = 1.0
        im = dict(shared)
        im.update(ein=ein.astype(f16), relw=relw,
                  invc=_pack_cols(invc_e, T), xT2=xT2.astype(f16))
        in_maps.append(im)

    # ---------------- build + run ------------------------------------------
    key = (EPAD, tuple(win_tiles))
    if key not in _CACHE:
        _CACHE[key] = _build_module(EPAD, win_tiles, h2_bufs)
    nc, names = _CACHE[key]

    from concourse import bass_utils
    trace = bool(int(os.environ.get("KERNEL_TRACE", "0")))
    if trace:
        try:
            import types
            import antenv
            if not hasattr(antenv, "axon_hooks"):
                mod = types.ModuleType("antenv.axon_hooks")
                mod._hook = None
                mod.set_axon_ntff_profile_hook = lambda h: setattr(mod, "_hook", h)
                mod.get_axon_ntff_profile_hook = lambda: mod._hook
                sys.modules["antenv.axon_hooks"] = mod
                antenv.axon_hooks = mod
                from trn_agent_boot.trn_boot import _ntff_profile_via_ctypes
                mod._hook = _ntff_profile_via_ctypes("/opt/axon/libaxon_pjrt.so")
        except Exception as e:  # profiling is best-effort
            print("ntff hook shim failed:", e)
            trace = False
    real_maps = [{names[k]: v for k, v in im.items()} for im in in_maps]
    res = bass_utils.run_bass_kernel_spmd(
        nc, real_maps, core_ids=list(range(NC)), trace=trace)
    if trace and res.exec_time_ns is not None:
        print(f"HW exec time: {res.exec_time_ns} ns")
        if res.instructions_and_trace:
            print("trace:", res.instructions_and_trace[1])

    out = np.empty(N_NODES, f32)
    for c in range(NC):
        out[NPN * c:NPN * (c + 1)] = res.results[c][names["zout"]][0, :NPN]
    return out


# revision 25
# speedup vs baseline: 1.2326x; 1.0137x over previous
"""OGRENet GNN message-passing kernel for 8 Trainium2 NeuronCores.

Strategy
--------
Host (numpy, cheap index plumbing only):
  * u2 = u @ Wsel + bsel (64x256), G64 = u2 @ eW1[19:275] (64x1024),
    H64 = u2 @ n2W1[521:777] (64x512): the per-edge/node u2 gather becomes a
    64-wide one-hot riding inside the K=128 first-layer matmul.
  * W_combo = eW5 @ n1W1[9:521]: e5 is never materialized (it is only ever
    consumed linearly by node MLP1), fusing two layers into one.
  * sort edges by destination node (`row`), split into 8 contiguous chunks at
    node boundaries -> each core owns a contiguous node range and ALL edges
    that scatter into it => no cross-core reduction.
  * pack per-core feature-major edge inputs ein = [x[col]; x[row]; e_attr;
    onehot(g); 1] (fp16, 128 rows total), per-edge 1/count scales, and
    window-relative row ids.

Device (per core, identical program, different data):
  * edge MLP layers as fp8 DoubleRow matmuls (157 TF/s), fp32 PSUM. Biases and
    relu are applied in a scale-staged way so the quantize ops spread across
    ACT/DVE/GpSimd and the tensor engine never waits:
      a1 = relu(ps)            x1   ACT   (bias via ones-row in ein)
      a2 = max(ps + 64 b2, 0)  x64  DVE   (2-op tensor_scalar)
      a3 = relu(ps/4096 + b3)  x1   ACT
      a4 = max(ps + 64 b4, 0)  x64  DVE
      h1 = relu(ps/64)         x64  ACT   (bias via ones-row in x-part)
      h2 = max((ps+4096 b) * invc/64, 0) fp8 x64  GpSimd add + DVE mul/relu
  * segment-sum as matmul with fp8 DoubleRow: membership pairs M[e,n]
    (is_equal on DVE) contract TWO 128-edge tiles per pass into a
    [feat, 256-node] PSUM window.
  * node MLP2 consumes agg feature-major; u2[batch] + bias enter through
    one-hot/ones rows of the x input tile; z -> DRAM.
"""

import os
import sys

import numpy as np

sys.path.insert(0, "/opt/trn_rl_repo")

N_NODES = 20000
N_GRAPHS = 64
E_HID = 1024
N_HID = 512
NC = 8
NPN = N_NODES // NC          # nodes per core (2500)
NPAD = 2560                  # padded nodes per core
WN = 256                     # nodes per segment window
NWIN = NPAD // WN            # 10
P = 128

_CACHE = {}


def _pack_cols(v, T):
    """[T*128] -> [128, T] with col t = v[t*128:(t+1)*128]."""
    return np.ascontiguousarray(v.reshape(T, P).T)


def _build_module(EPAD, win_tiles, h2_bufs):
    """Build the per-core Bass program. win_tiles[w] = (tlo, thi) inclusive."""
    from concourse import bacc, mybir, tile

    T = EPAD // P           # 128-edge tiles
    NB = EPAD // 1024       # 1024-edge blocks
    win_pairs = [(tl // 2, th // 2) for tl, th in win_tiles]
    f16 = mybir.dt.float16
    f32 = mybir.dt.float32
    f8 = mybir.dt.float8e4
    RELU = mybir.ActivationFunctionType.Relu
    COPY = mybir.ActivationFunctionType.Copy
    ADD = mybir.AluOpType.add
    MULT = mybir.AluOpType.mult
    MAX = mybir.AluOpType.max
    ISEQ = mybir.AluOpType.is_equal
    DR = mybir.MatmulPerfMode.DoubleRow

    nc = bacc.Bacc(None, target_bir_lowering=False, debug=False)

    with tile.TileContext(nc) as tc:
        with (
            tc.tile_pool(name="dram", bufs=1, space="DRAM") as dram,
            tc.tile_pool(name="wres", bufs=1) as wres,
            tc.tile_pool(name="einp", bufs=4) as einp,
            tc.tile_pool(name="actp", bufs=14) as actp,
            tc.tile_pool(name="h2p", bufs=h2_bufs) as h2p,
            tc.tile_pool(name="smal", bufs=8) as smal,
            tc.tile_pool(name="aggp", bufs=12) as aggp,
            tc.tile_pool(name="membp", bufs=24) as membp,
            tc.tile_pool(name="bigps", bufs=6, space="PSUM") as bigps,
            tc.tile_pool(name="segps", bufs=1, space="PSUM") as segps,
        ):
            # ---- DRAM I/O -------------------------------------------------
            d_ein = dram.tile([P, EPAD], f16, kind="ExternalInput", name="ein")
            d_relw = dram.tile([P, NWIN * T], f32, kind="ExternalInput", name="relw")
            d_invc = dram.tile([P, T], f32, kind="ExternalInput", name="invc")
            d_xT2 = dram.tile([P, NPAD], f16, kind="ExternalInput", name="xT2")
            d_iota = dram.tile([P, WN], f32, kind="ExternalInput", name="iota")
            d_n1b2bc = dram.tile([P, N_HID], f32, kind="ExternalInput", name="n1b2bc")

            wspec16 = dict(eW1p=[P, E_HID], n1W1a=[P, N_HID], n2W1x=[P, N_HID],
                           n2W2=[N_HID, 1])
            d_w = {k: dram.tile(s, f16, kind="ExternalInput", name=k)
                   for k, s in wspec16.items()}
            for k in ("eW2", "eW3", "eW4"):
                d_w[k] = dram.tile([P, 8192], f8, kind="ExternalInput", name=k)
            d_w["wcombo"] = dram.tile([P, 4096], f8, kind="ExternalInput", name="wcombo")
            d_w["n1W2"] = dram.tile([P, 2048], f8, kind="ExternalInput", name="n1W2")
            d_w["n2W1agg"] = dram.tile([P, 2048], f8, kind="ExternalInput",
                                       name="n2W1agg")
            bspec = dict(eb2r64=[P, 8], eb3r=[P, 8], eb4r64=[P, 8], n2b2r=[1, 1])
            d_b = {k: dram.tile(s, f32, kind="ExternalInput", name=k)
                   for k, s in bspec.items()}
            d_z = dram.tile([1, NPAD], f32, kind="ExternalOutput", name="zout")

            names = dict(ein=d_ein.name, relw=d_relw.name, invc=d_invc.name,
                         xT2=d_xT2.name, iota=d_iota.name, n1b2bc=d_n1b2bc.name,
                         zout=d_z.name)
            names.update({k: v.name for k, v in d_w.items()})
            names.update({k: v.name for k, v in d_b.items()})

            # ---- streamed ein blocks (SP queue, blocks 0/1 hoisted first) --
            ein_tiles = {}

            def fetch_ein(b):
                t = einp.tile([P, 1024], f16, name="ein_t")
                nc.sync.dma_start(out=t[:], in_=d_ein[:, b * 1024:(b + 1) * 1024])
                ein_tiles[b] = t

            fetch_ein(0)
            fetch_ein(1)

            # ---- resident loads --------------------------------------------
            # The ACT and DVE queues must stay free of DMA issues: an issue
            # whose HWDGE ring slot stalls behind bulk transfers blocks the
            # whole in-order queue, starving PE of quantized PSUM bufs. So
            # ACT issues only W1 (needed first, tiny); everything else rides
            # the SP and Pool queues, round-robin in compute-need order.
            W1 = wres.tile([P, E_HID], f16, name="w_eW1p")
            nc.scalar.dma_start(out=W1[:], in_=d_w["eW1p"][:])

            # tiny early-needed tensors first (negligible bytes)
            B = {}
            for k, s in bspec.items():
                t = wres.tile(s, f32, name=f"b_{k}")
                nc.sync.dma_start(out=t[:], in_=d_b[k][:])
                B[k] = t
            invc = wres.tile([P, T], f32, name="invc_sb")
            nc.gpsimd.dma_start(out=invc[:], in_=d_invc[:])
            n1b2bc = wres.tile([P, N_HID], f32, name="n1b2bc_sb")
            nc.gpsimd.dma_start(out=n1b2bc[:], in_=d_n1b2bc[:])
            iota = wres.tile([P, WN], f32, name="iota_sb")
            nc.gpsimd.dma_start(out=iota[:], in_=d_iota[:])

            qrr = [nc.gpsimd, nc.sync]
            qn = [0]

            def nextq():
                qn[0] += 1
                return qrr[qn[0] % 2]

            def load_wdr(name, npair, width):
                ts = []
                for q in range(npair):
                    t = wres.tile([P, 2, width], f8, name=f"w_{name}_{q}")
                    nextq().dma_start(
                        out=t[:, :, :],
                        in_=d_w[name][:, q * 2 * width:(q + 1) * 2 * width])
                    ts.append(t)
                return ts

            W2 = load_wdr("eW2", 4, E_HID)
            W3 = load_wdr("eW3", 4, E_HID)
            W4 = load_wdr("eW4", 4, E_HID)
            Wc = load_wdr("wcombo", 4, N_HID)
            W7 = load_wdr("n1W2", 2, N_HID)
            W6a = wres.tile([P, N_HID], f16, name="w_n1W1a")
            nc.gpsimd.dma_start(out=W6a[:], in_=d_w["n1W1a"][:])
            # Deferred loads (first needed by window 0, ~45us in): emitted on
            # the Pool queue behind a gate op so their ~2MB of HBM traffic
            # does not compete with the critical W2..W7 transfers. The gate
            # input is filled in during block 0 emission.
            W8x = wres.tile([P, N_HID], f16, name="w_n2W1x")
            W8a = [wres.tile([P, 2, N_HID], f8, name=f"w_n2W1agg_{q}")
                   for q in range(2)]
            W9 = [wres.tile([P, 1], f16, name=f"w_n2W2_{k}") for k in range(4)]
            relw = wres.tile([P, NWIN * T], f32, name="relw_sb")
            xT2 = wres.tile([P, NPAD], f16, name="xT2_sb")
            gdummy = wres.tile([1, 1], f16, name="gdummy")

            def emit_deferred_loads(gate_src):
                nc.gpsimd.tensor_scalar(out=gdummy[:], in0=gate_src,
                                        scalar1=0.0, scalar2=None, op0=ADD)
                nc.gpsimd.dma_start(out=W8x[:], in_=d_w["n2W1x"][:])
                for q in range(2):
                    nc.gpsimd.dma_start(
                        out=W8a[q][:, :, :],
                        in_=d_w["n2W1agg"][:, q * 1024:(q + 1) * 1024])
                for k in range(4):
                    nc.gpsimd.dma_start(out=W9[k][:],
                                        in_=d_w["n2W2"][k * P:(k + 1) * P, :])
                nc.gpsimd.dma_start(out=relw[:], in_=d_relw[:])
                nc.gpsimd.dma_start(out=xT2[:], in_=d_xT2[:])

            # h2 pair tiles by pair index (edge tiles 2k, 2k+1)
            h2_pairs = {}

            def dr_layer(pin, Wp, quant, tag):
                """fp8 DoubleRow layer; quant(out_plane, psum, m) applies the
                bias+relu+store for m-tile m."""
                outs = []
                pt = None
                for m in range(8):
                    ps = bigps.tile([P, 512], mybir.dt.float32, name="ps_big")
                    for q in range(len(Wp)):
                        nc.tensor.matmul(
                            out=ps[:], lhsT=Wp[q][:, :, m * P:(m + 1) * P],
                            rhs=pin[q][:, :, :], start=(q == 0),
                            stop=(q == len(Wp) - 1), perf_mode=DR)
                    if m % 2 == 0:
                        pt = actp.tile([P, 2, 512], f8, name=tag)
                        outs.append(pt)
                    quant(pt[:, m % 2, :], ps, m)
                return outs

            def q_dve64(btile):
                def q(o, ps, m):
                    nc.vector.tensor_scalar(
                        out=o, in0=ps[:], scalar1=btile[:, m:m + 1], scalar2=0.0,
                        op0=ADD, op1=MAX)
                return q

            def q_act(btile, scale):
                def q(o, ps, m):
                    nc.scalar.activation(o, ps[:], RELU,
                                         bias=btile[:, m:m + 1], scale=scale)
                return q

            def emit_block(b):
                if b + 2 < NB:
                    fetch_ein(b + 2)
                ein_t = ein_tiles.pop(b)
                for h in range(2):
                    einh = ein_t[:, h * 512:(h + 1) * 512]
                    # L1 -> a1 (x1) on ACT; bias rides the ones-row of ein
                    a1p = []
                    pt = None
                    for m in range(8):
                        ps = bigps.tile([P, 512], mybir.dt.float32, name="ps_big")
                        nc.tensor.matmul(out=ps[:], lhsT=W1[:, m * P:(m + 1) * P],
                                         rhs=einh, start=True, stop=True)
                        if m % 2 == 0:
                            pt = actp.tile([P, 2, 512], f8, name="pairA")
                            a1p.append(pt)
                        nc.scalar.activation(pt[:, m % 2, :], ps[:], RELU)
                    # L2 -> a2 (x64) on DVE
                    a2p = dr_layer(a1p, W2, q_dve64(B["eb2r64"]), "pairB")
                    # L3 -> a3 (x1) on ACT
                    a3p = dr_layer(a2p, W3, q_act(B["eb3r"], 1.0 / 4096), "pairA")
                    # L4 -> a4 (x64) on DVE
                    a4p = dr_layer(a3p, W4, q_dve64(B["eb4r64"]), "pairB")
                    # combo (eW5 @ n1W1 fused) + fp16 x-part -> h1 (x64) on ACT
                    h1p = []
                    pt = None
                    for m in range(4):
                        ps = bigps.tile([P, 512], mybir.dt.float32, name="ps_big")
                        nc.tensor.matmul(out=ps[:], lhsT=W6a[:, m * P:(m + 1) * P],
                                         rhs=einh, start=True, stop=False)
                        for q in range(4):
                            nc.tensor.matmul(
                                out=ps[:], lhsT=Wc[q][:, :, m * P:(m + 1) * P],
                                rhs=a4p[q][:, :, :], start=False, stop=(q == 3),
                                perf_mode=DR)
                        if m % 2 == 0:
                            pt = actp.tile([P, 2, 512], f8, name="pairC")
                            h1p.append(pt)
                        nc.scalar.activation(pt[:, m % 2, :], ps[:], RELU,
                                             scale=1.0 / 64)
                    # n1 L2: edge-major h2 per 128-edge subtile -> fp8 pairs
                    for s in range(4):
                        t_glob = b * 8 + h * 4 + s
                        ps = bigps.tile([P, 512], mybir.dt.float32, name="ps_big")
                        for q in range(2):
                            nc.tensor.matmul(
                                out=ps[:], lhsT=h1p[q][:, :, s * P:(s + 1) * P],
                                rhs=W7[q][:, :, :], start=(q == 0), stop=(q == 1),
                                perf_mode=DR)
                        tmp = smal.tile([P, 512], f16, name="tmp16")
                        nc.vector.tensor_tensor(out=tmp[:], in0=ps[:],
                                                in1=n1b2bc[:], op=ADD)
                        if b == 0 and h == 1 and s == 0:
                            gate_ref[0] = tmp[0:1, 0:1]
                        k, j = t_glob // 2, t_glob % 2
                        if j == 0:
                            h2_pairs[k] = h2p.tile([P, 2, N_HID], f8, name="h2t")
                        nc.vector.tensor_scalar(
                            out=h2_pairs[k][:, j, :], in0=tmp[:],
                            scalar1=invc[:, t_glob:t_glob + 1], scalar2=0.0,
                            op0=MULT, op1=MAX)

            win_agg = {}
            win_memb = {}

            def emit_window_memb(w):
                # membership depends only on resident iota/relw, so it is
                # staged one block early: DVE drains these during its slack
                # instead of delaying the next block's quantizes.
                klo, khi = win_pairs[w]
                ms = []
                for k in range(klo, khi + 1):
                    memb = membp.tile([P, 2, WN], f8, name="memb")
                    for j in range(2):
                        t = 2 * k + j
                        nc.vector.tensor_scalar(
                            out=memb[:, j, :], in0=iota[:],
                            scalar1=relw[:, w * T + t:w * T + t + 1], scalar2=None,
                            op0=ISEQ)
                    ms.append(memb)
                win_memb[w] = ms

            def emit_window_seg(w):
                klo, khi = win_pairs[w]
                seg = segps.tile([P, 4, WN], mybir.dt.float32, name="segps_t")
                pl = list(range(klo, khi + 1))
                ms = win_memb.pop(w)
                for ki_, k in enumerate(pl):
                    for fc in range(4):
                        nc.tensor.matmul(
                            out=seg[:, fc, :],
                            lhsT=h2_pairs[k][:, :, fc * P:(fc + 1) * P],
                            rhs=ms[ki_][:, :, :], start=(ki_ == 0),
                            stop=(ki_ == len(pl) - 1), perf_mode=DR)
                agg = []
                for p in range(2):
                    at = aggp.tile([P, 2, WN], f8, name="aggt")
                    agg.append(at)
                    for j in range(2):
                        nc.scalar.activation(at[:, j, :], seg[:, 2 * p + j, :], COPY)
                win_agg[w] = agg

            def emit_window_mlp(w):
                agg = win_agg.pop(w)
                # ---- node MLP2 on this 256-node window ----
                n0 = w * WN
                z1 = []
                for m in range(4):
                    ps = bigps.tile([P, 512], mybir.dt.float32, name="ps_big")
                    pss = ps[:, :WN]
                    nc.tensor.matmul(out=pss, lhsT=W8x[:, m * P:(m + 1) * P],
                                     rhs=xT2[:, n0:n0 + WN], start=True, stop=False)
                    for q in range(2):
                        nc.tensor.matmul(out=pss,
                                         lhsT=W8a[q][:, :, m * P:(m + 1) * P],
                                         rhs=agg[q][:, :, :], start=False,
                                         stop=(q == 1), perf_mode=DR)
                    zt = smal.tile([P, WN], f16, name="z1t")
                    nc.scalar.activation(zt[:], pss, RELU, scale=1.0 / 4096)
                    z1.append(zt)
                ps = bigps.tile([P, 512], mybir.dt.float32, name="ps_big")
                pss = ps[:1, :WN]
                for ki in range(4):
                    nc.tensor.matmul(out=pss, lhsT=W9[ki][:], rhs=z1[ki][:],
                                     start=(ki == 0), stop=(ki == 3))
                zo = smal.tile([1, WN], f32, name="zot")
                nc.vector.tensor_scalar(out=zo[:], in0=pss,
                                        scalar1=B["n2b2r"][0:1, 0:1], scalar2=None,
                                        op0=ADD)
                nc.sync.dma_start(out=d_z[:, n0:n0 + WN], in_=zo[:])

            # window w's h2 pairs complete during block win_tiles[w][1]//8;
            # its seg matmuls queue one block later (so the DVE h2 quantizes
            # have drained), the mlp2 part one block after that (so PE has
            # block work while ACT copies agg out).
            seg_at = {}
            memb_at = {}
            for w in range(NWIN):
                br = min(NB - 1, win_tiles[w][1] // 8)
                sb = br + 1 if br + 1 <= NB - 2 else br
                seg_at.setdefault(sb, []).append(w)
                memb_at.setdefault(max(0, sb - 1), []).append(w)
            gate_ref = [None]
            mlp_at = {}
            post_mlp = []
            for b in range(NB):
                emit_block(b)
                if b == 0:
                    emit_deferred_loads(gate_ref[0])
                for w in mlp_at.pop(b, []):
                    emit_window_mlp(w)
                for w in memb_at.pop(b, []):
                    emit_window_memb(w)
                for w in seg_at.get(b, []):
                    emit_window_seg(w)
                    if b + 1 < NB:
                        mlp_at.setdefault(b + 1, []).append(w)
                    else:
                        post_mlp.append(w)
            for w in post_mlp:
                emit_window_mlp(w)

    nc.compile()
    return nc, names


def kernel(x, edge_attr, u, edge_index, batch, Wsel, bsel,
           eW1, eb1, eW2, eb2, eW3, eb3, eW4, eb4, eW5, eb5,
           n1W1, n1b1, n1W2, n1b2, n2W1, n2b1, n2W2, n2b2):
    f32 = np.float32
    f16 = np.float16
    x = np.asarray(x, f32)
    edge_attr = np.asarray(edge_attr, f32)
    u = np.asarray(u, f32)
    edge_index = np.asarray(edge_index)
    batch = np.asarray(batch)
    ws = {k: np.asarray(v, f32) for k, v in dict(
        Wsel=Wsel, bsel=bsel, eW1=eW1, eb1=eb1, eW2=eW2, eb2=eb2, eW3=eW3,
        eb3=eb3, eW4=eW4, eb4=eb4, eW5=eW5, eb5=eb5, n1W1=n1W1, n1b1=n1b1,
        n1W2=n1W2, n1b2=n1b2, n2W1=n2W1, n2b1=n2b1, n2W2=n2W2, n2b2=n2b2).items()}

    # ---------------- host math (index plumbing + tiny matmuls) ------------
    u2 = (u @ ws["Wsel"] + ws["bsel"]).astype(f32)          # [64, 256]
    row = np.asarray(edge_index[0], np.int64)
    col = np.asarray(edge_index[1], np.int64)
    order = np.argsort(row, kind="stable")
    row_s, col_s = row[order], col[order]
    g_s = np.asarray(batch[row_s], np.int64)
    ea_s = edge_attr[order, 0]
    cnt = np.bincount(row, minlength=N_NODES).astype(f32)
    invc_node = (1.0 / np.maximum(cnt, 1.0)).astype(f32)

    bounds = np.searchsorted(row_s, np.arange(0, N_NODES + 1, NPN))
    e_cnt = np.diff(bounds)
    EPAD = int(-(-int(e_cnt.max()) // 1024) * 1024)
    T = EPAD // P

    # per-window tile ranges (shared across cores) for the static program
    tlo = np.full(NWIN, T - 1, np.int64)
    thi = np.zeros(NWIN, np.int64)
    core_dat = []
    for c in range(NC):
        lo, hi = bounds[c], bounds[c + 1]
        n = hi - lo
        rel = np.full(EPAD, 1e6, f32)
        rel[:n] = (row_s[lo:hi] - NPN * c).astype(f32)
        w_of_edge = np.floor_divide(rel[:n].astype(np.int64), WN)
        for w in range(NWIN):
            idx = np.nonzero(w_of_edge == w)[0]
            if idx.size:
                tlo[w] = min(tlo[w], idx[0] // P)
                thi[w] = max(thi[w], idx[-1] // P)
        core_dat.append((lo, hi, n, rel))
    win_tiles = [(int(tlo[w]), int(max(tlo[w], thi[w]))) for w in range(NWIN)]
    h2_bufs = max(th // 2 - tl // 2 + 1 for tl, th in win_tiles) + 18

    # ---------------- shared weight packing ---------------------------------
    G64 = u2 @ ws["eW1"][19:275]                      # [64, 1024]
    eW1p = np.zeros((P, E_HID), f32)
    eW1p[0:9] = ws["eW1"][9:18]       # x[col] (dest)
    eW1p[9:18] = ws["eW1"][0:9]       # x[row] (src)
    eW1p[18] = ws["eW1"][18]          # edge_attr
    eW1p[19:83] = G64                 # one-hot graph -> u2 @ eW1
    eW1p[83] = ws["eb1"]              # ones-row bias

    Wc_mat = ws["eW5"] @ ws["n1W1"][9:521]            # [1024, 512]
    b_combo = ws["eb5"] @ ws["n1W1"][9:521] + ws["n1b1"]
    n1W1a = np.zeros((P, N_HID), f32)
    n1W1a[0:9] = ws["n1W1"][0:9] * 4096.0
    n1W1a[83] = b_combo * 4096.0

    H64 = u2 @ ws["n2W1"][521:777]                    # [64, 512]
    n2W1x = np.zeros((P, N_HID), f32)
    n2W1x[0:9] = ws["n2W1"][0:9]
    n2W1x[9:73] = H64
    n2W1x[73] = ws["n2b1"]
    n2W1x *= 4096.0

    def br(b, nm):   # bias [nm*128] -> [128, nm]
        return np.ascontiguousarray(b.reshape(nm, P).T).astype(f32)

    import ml_dtypes
    fp8 = ml_dtypes.float8_e4m3

    def packdr(W):   # [K, M] -> [128, (K//128)*M] fp8, x64, (q,j,m) free order
        K, M = W.shape
        Wp = (W * 64.0).reshape(K // 256, 2, P, M)
        return np.ascontiguousarray(
            np.transpose(Wp, (2, 0, 1, 3)).reshape(P, (K // P) * M)).astype(fp8)

    shared = dict(
        eW1p=eW1p.astype(f16), eW2=packdr(ws["eW2"]),
        eW3=packdr(ws["eW3"]), eW4=packdr(ws["eW4"]),
        wcombo=packdr(Wc_mat), n1W2=packdr(ws["n1W2"]),
        n1W1a=n1W1a.astype(f16), n2W1x=n2W1x.astype(f16),
        n2W1agg=packdr(ws["n2W1"][9:521]),
        n2W2=ws["n2W2"].astype(f16),
        eb2r64=br(ws["eb2"] * 64.0, 8), eb3r=br(ws["eb3"], 8),
        eb4r64=br(ws["eb4"] * 64.0, 8),
        n2b2r=ws["n2b2"].reshape(1, 1).astype(f32),
        iota=np.tile(np.arange(WN, dtype=f32), (P, 1)),
        n1b2bc=np.tile(ws["n1b2"].astype(f32) * 4096.0, (P, 1)),
    )

    # ---------------- per-core input packing --------------------------------
    in_maps = []
    for c in range(NC):
        lo, hi, n, rel = core_dat[c]
        ein = np.zeros((P, EPAD), f32)
        ein[0:9, :n] = x[col_s[lo:hi]].T
        ein[9:18, :n] = x[row_s[lo:hi]].T
        ein[18, :n] = ea_s[lo:hi]
        ein[19 + g_s[lo:hi], np.arange(n)] = 1.0
        ein[83, :n] = 1.0
        relw = np.empty((P, NWIN * T), f32)
        for w in range(NWIN):
            relw[:, w * T:(w + 1) * T] = _pack_cols(rel - float(WN) * w, T)
        invc_e = np.ones(EPAD, f32)
        invc_e[:n] = invc_node[row_s[lo:hi]]
        invc_e *= 1.0 / 64.0
        xT2 = np.zeros((P, NPAD), f32)
        xT2[0:9, :NPN] = x[NPN * c:NPN * (c + 1)].T
        bc = np.asarray(batch[NPN * c:NPN * (c + 1)], np.int64)
        xT2[9 + bc, np.arange(NPN)] = 1.0
        xT2[73, :NPN] = 1.0
        im = dict(shared)
        im.update(ein=ein.astype(f16), relw=relw,
                  invc=_pack_cols(invc_e, T), xT2=xT2.astype(f16))
        in_maps.append(im)

    # ---------------- build + run ------------------------------------------
    key = (EPAD, tuple(win_tiles))
    if key not in _CACHE:
        _CACHE[key] = _build_module(EPAD, win_tiles, h2_bufs)
    nc, names = _CACHE[key]

    from concourse import bass_utils
    trace = bool(int(os.environ.get("KERNEL_TRACE", "0")))
    if trace:
        try:
            import types
            import antenv
            if not hasattr(antenv, "axon_hooks"):
                mod = types.ModuleType("antenv.axon_hooks")
                mod._hook = None
                mod.set_axon_ntff_profile_hook = lambda h: setattr(mod, "_hook", h)
                mod.get_axon_ntff_profile_hook = lambda: mod._hook
                sys.modules["antenv.axon_hooks"] = mod
                antenv.axon_hooks = mod
                from trn_agent_boot.trn_boot import _ntff_profile_via_ctypes
                mod._hook = _ntff_profile_via_ctypes("/opt/axon/libaxon_pjrt.so")
        except Exception as e:  # profiling is best-effort
            print("ntff hook shim failed:", e)
            trace = False
    real_maps = [{names[k]: v for k, v in im.items()} for im in in_maps]
    res = bass_utils.run_bass_kernel_spmd(
        nc, real_maps, core_ids=list(range(NC)), trace=trace)
    if trace and res.exec_time_ns is not None:
        print(f"HW exec time: {res.exec_time_ns} ns")
        if res.instructions_and_trace:
            print("trace:", res.instructions_and_trace[1])

    out = np.empty(N_NODES, f32)
    for c in range(NC):
        out[NPN * c:NPN * (c + 1)] = res.results[c][names["zout"]][0, :NPN]
    return out
